# revision 2
# baseline (speedup 1.0000x reference)
"""Trainium2 Bass kernel for NeoX-style attention block (B=2, S=2048, D=2048,
H=16, HS=128, partial RoPE rot=32, no mask) sharded over 8 NeuronCores.

Sharding: core c handles batch b = c//4 and head group g = c%4 (4 heads).
Tensor-parallel over heads: W_qkv column-sliced, W_dense row-sliced; each core
produces a partial [S, D] output; host sums 4 partials per batch + bias.

All matmul operands are bf16 (f32 PSUM accumulation): identical PE rate to
f32r at N>=256 but halves DMA/SBUF, enables Fast Weight Load, and doubles DVE
throughput for 16-bit elementwise ops.  Single QKV pass (weights resident).

Per-core dataflow:
  q,k produced transposed (qkT[hs, tok]) via lhsT = W chunks, rhs = hT chunks;
  V produced NATURAL ([tok, hs]) via the swapped matmul lhsT = hT chunk,
  rhs = Wv (kills the per-head PE transposes of V).  Partial RoPE (first 32
  rows of each q/k chunk) via a rotate-half matmul + 3 DVE ops per window.
  Scores^T chunks S^T[k, q] = K^T_chunk.T @ Q^T (512-wide), exp on ACT
  (bf16 out), AV accum O^T = V_chunk.T @ E plus a ones-matmul accumulating
  softmax denominators; normalization = reciprocal_approx_fast + multiply.
  Dense partial uses lhsT = O^T chunks, rhs = W_dense row-slice.
"""
import sys

sys.path.insert(0, "/opt/trn_rl_repo")

import numpy as np
import ml_dtypes
from contextlib import ExitStack

import concourse.bass as bass  # noqa: F401  (registers engine types)
import concourse.tile as tile
from concourse import bacc, mybir
from concourse import bass_utils

F32 = mybir.dt.float32
BF16 = mybir.dt.bfloat16
NPBF = ml_dtypes.bfloat16
MUL = mybir.AluOpType.mult
ADD = mybir.AluOpType.add

B, S, D = 2, 2048, 2048
H, HS, ROT = 16, 128, 32
BASE = 10000.0
SM_SCALE = 1.0 / float(np.sqrt(HS))

HPC = 4            # heads per core
CPB = 4            # cores per batch
NCORES = 8
KC = D // 128      # 16 contraction chunks
NW = 4             # token windows of 512

_NC = None
TRACE = False
LAST_RESULT = [None]


def _build():
    nc = bacc.Bacc("TRN2", target_bir_lowering=False, debug=False)
    h16d = nc.dram_tensor("h16d", [D, S], BF16, kind="ExternalInput").ap()
    wqk16 = nc.dram_tensor("wqk16", [D, 8 * 128], BF16, kind="ExternalInput").ap()
    wv16 = nc.dram_tensor("wv16", [D, HPC * 128], BF16, kind="ExternalInput").ap()
    wd16 = nc.dram_tensor("wd16", [HPC * HS, D], BF16, kind="ExternalInput").ap()
    tabc16 = nc.dram_tensor("tabc16", [ROT, S], BF16, kind="ExternalInput").ap()
    tabs16 = nc.dram_tensor("tabs16", [ROT, S], BF16, kind="ExternalInput").ap()
    rotm16 = nc.dram_tensor("rotm16", [ROT, ROT], BF16, kind="ExternalInput").ap()
    ones16 = nc.dram_tensor("ones16", [128, 128], BF16, kind="ExternalInput").ap()
    bqk = nc.dram_tensor("bqk", [128, 8], F32, kind="ExternalInput").ap()
    bv = nc.dram_tensor("bv", [128, HPC * 128], F32, kind="ExternalInput").ap()
    outp = nc.dram_tensor("outp", [S, D], F32, kind="ExternalOutput").ap()

    with tile.TileContext(nc) as tc:
        with ExitStack() as ctx:
            glob = ctx.enter_context(tc.tile_pool(name="glob", bufs=1))
            hpool = ctx.enter_context(tc.tile_pool(name="hpool", bufs=2))
            epool = ctx.enter_context(tc.tile_pool(name="epool", bufs=3))
            bopool = ctx.enter_context(tc.tile_pool(name="bopool", bufs=4))
            ps = ctx.enter_context(tc.tile_pool(name="ps", bufs=1, space="PSUM"))

            # ---- resident weights / tables ----
            wqk_sb = glob.tile([128, 8 * KC * 128], BF16, tag="wqk")
            for m in range(8):
                nc.sync.dma_start(
                    wqk_sb[:, m * 2048:(m + 1) * 2048].rearrange(
                        "p (kc c) -> p kc c", kc=KC),
                    wqk16[:, m * 128:(m + 1) * 128].rearrange(
                        "(kc p) c -> p kc c", p=128),
                )
            wv_sb = glob.tile([128, KC * 512], BF16, tag="wv")
            nc.sync.dma_start(
                wv_sb.rearrange("p (kc c) -> p kc c", kc=KC),
                wv16.rearrange("(kc p) c -> p kc c", p=128),
            )
            tabc_sb = glob.tile([ROT, S], BF16, tag="tabc")
            nc.sync.dma_start(tabc_sb[:], tabc16)
            tabs_sb = glob.tile([ROT, S], BF16, tag="tabs")
            nc.sync.dma_start(tabs_sb[:], tabs16)
            rot_sb = glob.tile([ROT, ROT], BF16, tag="rotm")
            nc.sync.dma_start(rot_sb[:], rotm16)
            ones_sb = glob.tile([128, 128], BF16, tag="ones")
            nc.sync.dma_start(ones_sb[:], ones16)
            bqk_sb = glob.tile([128, 8], F32, tag="bqk")
            nc.sync.dma_start(bqk_sb[:], bqk)
            bv_sb = glob.tile([128, HPC * 128], F32, tag="bv")
            nc.sync.dma_start(bv_sb[:], bv)
            wd_sb = glob.tile([128, HPC * D], BF16, tag="wd")
            nc.sync.dma_start(
                wd_sb.rearrange("p (hc d) -> p hc d", hc=HPC),
                wd16.rearrange("(hc p) d -> p hc d", p=128),
            )

            # persistent activations
            qkT = glob.tile([128, 8 * S], BF16, tag="qkT")   # [hs, m*S + tok]
            vn = glob.tile([128, HPC * S], BF16, tag="vn")   # [ktok, h*S+kc*128+hs]
            oT = glob.tile([128, HPC * S], BF16, tag="oT")   # [hs, h*S + tok]

            def rope(m, n):
                # rotate first ROT dims of q/k chunk m for token window n
                sl = slice(m * S + n * 512, m * S + (n + 1) * 512)
                wsl = slice(n * 512, (n + 1) * 512)
                pr = ps.tile([128, 512], F32, tag="u", bufs=4, name=f"pr{m}_{n}")
                nc.tensor.matmul(pr[0:ROT, :], rot_sb[:, :], qkT[0:ROT, sl],
                                 start=True, stop=True)
                nc.vector.tensor_tensor(pr[0:ROT, :], pr[0:ROT, :],
                                        tabs_sb[:, wsl], op=MUL)
                nc.vector.tensor_tensor(qkT[0:ROT, sl], qkT[0:ROT, sl],
                                        tabc_sb[:, wsl], op=MUL)
                nc.vector.tensor_tensor(qkT[0:ROT, sl], qkT[0:ROT, sl],
                                        pr[0:ROT, :], op=ADD)

            # ---- QKV projection (single pass, all 4 heads) ----
            for n in range(NW):
                ht = hpool.tile([128, KC * 512], BF16, tag="ht")
                for kc in range(KC):
                    nc.sync.dma_start(
                        ht[:, kc * 512:(kc + 1) * 512],
                        h16d[kc * 128:(kc + 1) * 128, n * 512:(n + 1) * 512])
                for m in range(8):
                    pq = ps.tile([128, 512], F32, tag="u", bufs=4,
                                 name=f"pq{m}_{n}")
                    for kc in range(KC):
                        nc.tensor.matmul(
                            pq[:],
                            wqk_sb[:, m * 2048 + kc * 128:m * 2048 + (kc + 1) * 128],
                            ht[:, kc * 512:(kc + 1) * 512],
                            start=(kc == 0), stop=(kc == KC - 1))
                    nc.vector.tensor_scalar_add(
                        qkT[:, m * S + n * 512:m * S + (n + 1) * 512],
                        pq[:], bqk_sb[:, m:m + 1])
                    rope(m, n)
                for t4 in range(4):
                    pv = ps.tile([128, 512], F32, tag="u", bufs=4,
                                 name=f"pv{t4}_{n}")
                    for kc in range(KC):
                        nc.tensor.matmul(
                            pv[:],
                            ht[:, kc * 512 + t4 * 128:kc * 512 + (t4 + 1) * 128],
                            wv_sb[:, kc * 512:(kc + 1) * 512],
                            start=(kc == 0), stop=(kc == KC - 1))
                    tc4 = n * 4 + t4     # global 128-token chunk index
                    nc.vector.tensor_tensor(
                        vn.rearrange("p (h t) -> p h t", h=HPC)[
                            :, :, tc4 * 128:(tc4 + 1) * 128],
                        pv[:].rearrange("p (h t) -> p h t", h=HPC),
                        bv_sb.rearrange("p (h t) -> p h t", h=HPC),
                        op=ADD)

            # ---- attention, software-pipelined one exp behind ----
            prev = [None]

            def consume(h, qs, k2, pa, e):
                po, pm = pa[:, 0:512], pa[:, 512:1024]
                for j in range(2):
                    kc = 2 * k2 + j
                    nc.tensor.matmul(po, vn[:, h * S + kc * 128:h * S + (kc + 1) * 128],
                                     e[:, j * 512:(j + 1) * 512],
                                     start=(kc == 0), stop=(kc == KC - 1))
                for j in range(2):
                    kc = 2 * k2 + j
                    nc.tensor.matmul(pm, ones_sb[:], e[:, j * 512:(j + 1) * 512],
                                     start=(kc == 0), stop=(kc == KC - 1))
                if k2 == KC // 2 - 1:
                    rc = epool.tile([128, 512], F32, tag="rc", bufs=2,
                                    name=f"rc{h}_{qs}")
                    nc.vector.reciprocal_approx_fast(rc[:], pm)
                    nc.vector.tensor_tensor(
                        oT[:, h * S + qs * 512:h * S + (qs + 1) * 512],
                        po, rc[:], op=MUL)

            def attention_head(h):
                qoff, koff = (2 * h) * S, (2 * h + 1) * S
                for qs in range(4):
                    pa = ps.tile([128, 1024], F32, tag="u", bufs=4,
                                 name=f"pa{h}_{qs}")
                    for k2 in range(KC // 2):
                        pS = ps.tile([128, 1024], F32, tag="u", bufs=4,
                                     name=f"pS{h}_{qs}_{k2}")
                        for j in range(2):
                            kc = 2 * k2 + j
                            nc.tensor.matmul(
                                pS[:, j * 512:(j + 1) * 512],
                                qkT[:, koff + kc * 128:koff + (kc + 1) * 128],
                                qkT[:, qoff + qs * 512:qoff + (qs + 1) * 512],
                                start=True, stop=True)
                        e = epool.tile([128, 1024], BF16, tag="e")
                        nc.scalar.activation(e[:], pS[:],
                                             mybir.ActivationFunctionType.Exp)
                        if prev[0] is not None:
                            consume(*prev[0])
                        prev[0] = (h, qs, k2, pa, e)

            for h in range(HPC):
                attention_head(h)
            consume(*prev[0])

            # ---- dense partial ----
            for tt in range(S // 128):
                for dsp in range(4):
                    pd = ps.tile([128, 512], F32, tag="u", bufs=4,
                                 name=f"pd{tt}_{dsp}")
                    for hc in range(HPC):
                        nc.tensor.matmul(
                            pd[:],
                            oT[:, hc * S + tt * 128:hc * S + (tt + 1) * 128],
                            wd_sb[:, hc * D + dsp * 512:hc * D + (dsp + 1) * 512],
                            start=(hc == 0), stop=(hc == HPC - 1))
                    bo = bopool.tile([128, 512], F32, tag="bo")
                    if dsp % 2 == 0:
                        nc.scalar.copy(bo[:], pd[:])
                    else:
                        nc.vector.tensor_copy(bo[:], pd[:])
                    nc.sync.dma_start(
                        outp[tt * 128:(tt + 1) * 128, dsp * 512:(dsp + 1) * 512],
                        bo[:])
    nc.compile()
    return nc


def _rope_tables(position_ids_b):
    pos = np.asarray(position_ids_b, dtype=np.float64)
    inv_freq = 1.0 / (BASE ** (np.arange(0, ROT, 2, dtype=np.float64) / ROT))
    freqs = np.outer(pos, inv_freq)                       # [S, 16]
    emb = np.concatenate([freqs, freqs], axis=-1)         # [S, 32]
    return (np.cos(emb).T.astype(NPBF).copy(),
            np.sin(emb).T.astype(NPBF).copy())


def kernel(hidden_states, position_ids, W_qkv, b_qkv, W_dense, b_dense):
    global _NC
    if _NC is None:
        _NC = _build()
    nc = _NC

    hidden_states = np.asarray(hidden_states, dtype=np.float32)
    W_qkv = np.asarray(W_qkv, dtype=np.float32)
    b_qkv = np.asarray(b_qkv, dtype=np.float32)
    W_dense = np.asarray(W_dense, dtype=np.float32)
    b_dense = np.asarray(b_dense, dtype=np.float32)

    rotm = np.zeros((ROT, ROT), np.float32)
    half = ROT // 2
    for i in range(half):
        rotm[i + half, i] = -1.0
        rotm[i, i + half] = 1.0
    ones = np.ones((128, 128), np.float32)

    hTs = [np.ascontiguousarray(hidden_states[b].T).astype(NPBF) for b in range(B)]
    tabs_per_b = [_rope_tables(np.asarray(position_ids)[b]) for b in range(B)]

    in_maps = []
    for c in range(NCORES):
        b, g = divmod(c, CPB)
        # per-head column slices of W_qkv (NeoX fused layout: head-major,
        # [q(128) k(128) v(128)] per head)
        wqk = np.empty((D, 8 * 128), np.float32)
        wv = np.empty((D, HPC * 128), np.float32)
        bqk_host = np.empty((128, 8), np.float32)
        bv_host = np.empty((128, HPC * 128), np.float32)
        for hp in range(HPC):
            c0 = (g * HPC + hp) * 3 * HS
            wqk[:, (2 * hp) * 128:(2 * hp + 1) * 128] = \
                W_qkv[:, c0:c0 + HS] * SM_SCALE
            wqk[:, (2 * hp + 1) * 128:(2 * hp + 2) * 128] = \
                W_qkv[:, c0 + HS:c0 + 2 * HS]
            wv[:, hp * 128:(hp + 1) * 128] = W_qkv[:, c0 + 2 * HS:c0 + 3 * HS]
            bqk_host[:, 2 * hp] = b_qkv[c0:c0 + HS] * SM_SCALE
            bqk_host[:, 2 * hp + 1] = b_qkv[c0 + HS:c0 + 2 * HS]
            bv_host[:, hp * 128:(hp + 1) * 128] = \
                b_qkv[c0 + 2 * HS:c0 + 3 * HS][None, :]
        cosT, sinT = tabs_per_b[b]
        in_maps.append({
            "h16d": hTs[b],
            "wqk16": wqk.astype(NPBF),
            "wv16": wv.astype(NPBF),
            "wd16": np.ascontiguousarray(
                W_dense[g * HPC * HS:(g + 1) * HPC * HS, :]).astype(NPBF),
            "tabc16": cosT,
            "tabs16": sinT,
            "rotm16": rotm.astype(NPBF),
            "ones16": ones.astype(NPBF),
            "bqk": bqk_host,
            "bv": bv_host,
        })

    res = bass_utils.run_bass_kernel_spmd(
        nc, in_maps, core_ids=list(range(NCORES)), trace=TRACE)
    LAST_RESULT[0] = res

    out = np.empty((B, S, D), np.float32)
    for b in range(B):
        acc = np.zeros((S, D), np.float64)
        for g in range(CPB):
            acc += res.results[b * CPB + g]["outp"]
        out[b] = (acc + b_dense).astype(np.float32)
    return out


# revision 6
# speedup vs baseline: 1.1450x; 1.1450x over previous
"""Trainium2 Bass kernel for NeoX-style attention block (B=2, S=2048, D=2048,
H=16, HS=128, partial RoPE rot=32, no mask) sharded over 8 NeuronCores.

Sharding: core c handles batch b = c//4 and head group g = c%4 (4 heads).
Tensor-parallel over heads: W_qkv column-sliced, W_dense row-sliced; each core
produces a partial [S, D] output; host sums 4 partials per batch + bias.

All matmul operands are bf16 (f32 PSUM accumulation): identical PE rate to
f32r at N>=256 but halves DMA/SBUF, enables Fast Weight Load, and doubles DVE
throughput for 16-bit elementwise ops.  Single QKV pass (weights resident).

Per-core dataflow:
  q,k produced transposed (qkT[hs, tok]) via lhsT = W chunks, rhs = hT chunks;
  V produced NATURAL ([tok, hs]) via the swapped matmul lhsT = hT chunk,
  rhs = Wv (kills the per-head PE transposes of V).  Partial RoPE (first 32
  rows of each q/k chunk) via a rotate-half matmul + 3 DVE ops per window.
  Scores^T chunks S^T[k, q] = K^T_chunk.T @ Q^T (512-wide), exp on ACT
  (bf16 out), AV accum O^T = V_chunk.T @ E plus a ones-matmul accumulating
  softmax denominators; normalization = reciprocal_approx_fast + multiply.
  Dense partial uses lhsT = O^T chunks, rhs = W_dense row-slice.
"""
import sys

sys.path.insert(0, "/opt/trn_rl_repo")

import numpy as np
import ml_dtypes
from contextlib import ExitStack

import concourse.bass as bass  # noqa: F401  (registers engine types)
import concourse.tile as tile
from concourse import bacc, mybir
from concourse import bass_utils

F32 = mybir.dt.float32
BF16 = mybir.dt.bfloat16
NPBF = ml_dtypes.bfloat16
MUL = mybir.AluOpType.mult
ADD = mybir.AluOpType.add

B, S, D = 2, 2048, 2048
H, HS, ROT = 16, 128, 32
BASE = 10000.0
SM_SCALE = 1.0 / float(np.sqrt(HS))

HPC = 4            # heads per core
CPB = 4            # cores per batch
NCORES = 8
KC = D // 128      # 16 contraction chunks
NW = 4             # token windows of 512

_NC = None
TRACE = False
LAST_RESULT = [None]


def _build():
    nc = bacc.Bacc("TRN2", target_bir_lowering=False, debug=False)
    h16d = nc.dram_tensor("h16d", [D, S], BF16, kind="ExternalInput").ap()
    wqk16 = nc.dram_tensor("wqk16", [D, 8 * 128], BF16, kind="ExternalInput").ap()
    wv16 = nc.dram_tensor("wv16", [D, HPC * 128], BF16, kind="ExternalInput").ap()
    wd16 = nc.dram_tensor("wd16", [HPC * HS, D], BF16, kind="ExternalInput").ap()
    tabc16 = nc.dram_tensor("tabc16", [ROT, S], BF16, kind="ExternalInput").ap()
    tabs16 = nc.dram_tensor("tabs16", [ROT, S], BF16, kind="ExternalInput").ap()
    rotm16 = nc.dram_tensor("rotm16", [ROT, ROT], BF16, kind="ExternalInput").ap()
    ones16 = nc.dram_tensor("ones16", [128, 128], BF16, kind="ExternalInput").ap()
    bqk = nc.dram_tensor("bqk", [128, 8], F32, kind="ExternalInput").ap()
    bv = nc.dram_tensor("bv", [128, HPC * 128], F32, kind="ExternalInput").ap()
    outp = nc.dram_tensor("outp", [S, D], F32, kind="ExternalOutput").ap()

    with tile.TileContext(nc) as tc:
        with ExitStack() as ctx:
            glob = ctx.enter_context(tc.tile_pool(name="glob", bufs=1))
            hpool = ctx.enter_context(tc.tile_pool(name="hpool", bufs=2))
            epool = ctx.enter_context(tc.tile_pool(name="epool", bufs=3))
            bopool = ctx.enter_context(tc.tile_pool(name="bopool", bufs=4))
            ps = ctx.enter_context(tc.tile_pool(name="ps", bufs=1, space="PSUM"))

            # ---- resident weights / tables ----
            # DMA issue order is critical for startup latency: first QKV
            # weight chunk + first token window lead; dense weights trail.
            wqk_sb = glob.tile([128, 8 * KC * 128], BF16, tag="wqk")

            def load_wqk(m):
                nc.sync.dma_start(
                    wqk_sb[:, m * 2048:(m + 1) * 2048].rearrange(
                        "p (kc c) -> p kc c", kc=KC),
                    wqk16[:, m * 128:(m + 1) * 128].rearrange(
                        "(kc p) c -> p kc c", p=128),
                )

            load_wqk(0)

            def load_ht(n):
                ht = hpool.tile([128, KC * 512], BF16, tag="ht",
                                name=f"ht{n}")
                for k4 in range(4):
                    nc.sync.dma_start(
                        ht[:, k4 * 2048:(k4 + 1) * 2048].rearrange(
                            "p (kc s) -> p kc s", kc=4),
                        h16d[k4 * 512:(k4 + 1) * 512,
                             n * 512:(n + 1) * 512].rearrange(
                            "(kc p) s -> p kc s", p=128))
                return ht

            ht0 = load_ht(0)
            load_wqk(1)
            tabc_sb = glob.tile([ROT, S], BF16, tag="tabc")
            nc.sync.dma_start(tabc_sb[:], tabc16)
            tabs_sb = glob.tile([ROT, S], BF16, tag="tabs")
            nc.sync.dma_start(tabs_sb[:], tabs16)
            rot_sb = glob.tile([ROT, ROT], BF16, tag="rotm")
            nc.sync.dma_start(rot_sb[:], rotm16)
            bqk_sb = glob.tile([128, 8], F32, tag="bqk")
            nc.sync.dma_start(bqk_sb[:], bqk)
            for m in range(2, 8):
                load_wqk(m)
            wv_sb = glob.tile([128, KC * 512], BF16, tag="wv")
            nc.sync.dma_start(
                wv_sb.rearrange("p (kc c) -> p kc c", kc=KC),
                wv16.rearrange("(kc p) c -> p kc c", p=128),
            )
            bv_sb = glob.tile([128, HPC * 128], F32, tag="bv")
            nc.sync.dma_start(bv_sb[:], bv)
            ones_sb = glob.tile([128, 128], BF16, tag="ones")
            nc.sync.dma_start(ones_sb[:], ones16)

            # persistent activations
            qkT = glob.tile([128, 8 * S], BF16, tag="qkT")   # [hs, m*S + tok]
            vn = glob.tile([128, HPC * S], BF16, tag="vn")   # [ktok, h*S+kc*128+hs]
            oT = glob.tile([128, HPC * S], BF16, tag="oT")   # [hs, h*S + tok]
            wd_sb = glob.tile([128, HPC * D], BF16, tag="wd")

            def rope(m, n):
                # rotate first ROT dims of q/k chunk m for token window n.
                # Issued one m-chunk late so the rot matmul's dependency (the
                # DVE eviction of chunk m) is already done when PE reaches it.
                sl = slice(m * S + n * 512, m * S + (n + 1) * 512)
                wsl = slice(n * 512, (n + 1) * 512)
                pr = ps.tile([128, 512], F32, tag="v5", bufs=4, name=f"pr{m}_{n}")
                nc.tensor.matmul(pr[0:ROT, :], rot_sb[:, :], qkT[0:ROT, sl],
                                 start=True, stop=True)
                nc.vector.tensor_tensor(pr[0:ROT, :], pr[0:ROT, :],
                                        tabs_sb[:, wsl], op=MUL)
                nc.vector.tensor_tensor(qkT[0:ROT, sl], qkT[0:ROT, sl],
                                        tabc_sb[:, wsl], op=MUL)
                nc.vector.tensor_tensor(qkT[0:ROT, sl], qkT[0:ROT, sl],
                                        pr[0:ROT, :], op=ADD)

            # ---- QKV projection (single pass, all 4 heads) ----
            for n in range(NW):
                ht = ht0 if n == 0 else load_ht(n)
                for m in range(8):
                    pq = ps.tile([128, 512], F32, tag="v5", bufs=4,
                                 name=f"pq{m}_{n}")
                    for kc in range(KC):
                        nc.tensor.matmul(
                            pq[:],
                            wqk_sb[:, m * 2048 + kc * 128:m * 2048 + (kc + 1) * 128],
                            ht[:, kc * 512:(kc + 1) * 512],
                            start=(kc == 0), stop=(kc == KC - 1))
                    nc.vector.tensor_scalar_add(
                        qkT[:, m * S + n * 512:m * S + (n + 1) * 512],
                        pq[:], bqk_sb[:, m:m + 1])
                    if m > 0:
                        rope(m - 1, n)
                for t4 in range(4):
                    pv = ps.tile([128, 512], F32, tag="v5", bufs=4,
                                 name=f"pv{t4}_{n}")
                    for kc in range(KC):
                        nc.tensor.matmul(
                            pv[:],
                            ht[:, kc * 512 + t4 * 128:kc * 512 + (t4 + 1) * 128],
                            wv_sb[:, kc * 512:(kc + 1) * 512],
                            start=(kc == 0), stop=(kc == KC - 1))
                    tc4 = n * 4 + t4     # global 128-token chunk index
                    nc.vector.tensor_tensor(
                        vn.rearrange("p (h t) -> p h t", h=HPC)[
                            :, :, tc4 * 128:(tc4 + 1) * 128],
                        pv[:].rearrange("p (h t) -> p h t", h=HPC),
                        bv_sb.rearrange("p (h t) -> p h t", h=HPC),
                        op=ADD)
                    if n == NW - 1 and t4 == 0:
                        rope(7, n)
                if n < NW - 1:
                    rope(7, n)

            # ---- attention, software-pipelined one exp behind ----
            prev = [None]

            def consume(h, qs, k2, po, pm, e):
                for j in range(2):
                    kc = 2 * k2 + j
                    nc.tensor.matmul(po, vn[:, h * S + kc * 128:h * S + (kc + 1) * 128],
                                     e[:, j * 512:(j + 1) * 512],
                                     start=(kc == 0), stop=(kc == KC - 1))
                for j in range(2):
                    kc = 2 * k2 + j
                    nc.tensor.matmul(pm, ones_sb[:], e[:, j * 512:(j + 1) * 512],
                                     start=(kc == 0), stop=(kc == KC - 1))
                if k2 == KC // 2 - 1:
                    rc = epool.tile([128, 512], F32, tag="rc", bufs=2,
                                    name=f"rc{h}_{qs}")
                    nc.vector.reciprocal_approx_fast(rc[:], pm)
                    nc.vector.tensor_tensor(
                        oT[:, h * S + qs * 512:h * S + (qs + 1) * 512],
                        po, rc[:], op=MUL)

            def attention_head(h):
                qoff, koff = (2 * h) * S, (2 * h + 1) * S
                for qs in range(4):
                    po = ps.tile([128, 512], F32, tag="v5", bufs=4,
                                 name=f"po{h}_{qs}")
                    pm = ps.tile([128, 512], F32, tag="v5", bufs=4,
                                 name=f"pm{h}_{qs}")
                    for k2 in range(KC // 2):
                        pS = ps.tile([128, 1024], F32, tag="pS", bufs=2,
                                     name=f"pS{h}_{qs}_{k2}")
                        for j in range(2):
                            kc = 2 * k2 + j
                            nc.tensor.matmul(
                                pS[:, j * 512:(j + 1) * 512],
                                qkT[:, koff + kc * 128:koff + (kc + 1) * 128],
                                qkT[:, qoff + qs * 512:qoff + (qs + 1) * 512],
                                start=True, stop=True)
                        e = epool.tile([128, 1024], BF16, tag="e")
                        nc.scalar.activation(e[:], pS[:],
                                             mybir.ActivationFunctionType.Exp)
                        if prev[0] is not None:
                            consume(*prev[0])
                        prev[0] = (h, qs, k2, po, pm, e)
                if h == 0:
                    # dense weights: issued here so the transfer overlaps
                    # attention; needed only at the dense stage
                    nc.sync.dma_start(
                        wd_sb.rearrange("p (hc d) -> p hc d", hc=HPC),
                        wd16.rearrange("(hc p) d -> p hc d", p=128),
                    )

            for h in range(HPC):
                attention_head(h)
            consume(*prev[0])

            # ---- dense partial ----
            for tt in range(S // 128):
                for dsp in range(4):
                    pd = ps.tile([128, 512], F32, tag="v5", bufs=4,
                                 name=f"pd{tt}_{dsp}")
                    for hc in range(HPC):
                        nc.tensor.matmul(
                            pd[:],
                            oT[:, hc * S + tt * 128:hc * S + (tt + 1) * 128],
                            wd_sb[:, hc * D + dsp * 512:hc * D + (dsp + 1) * 512],
                            start=(hc == 0), stop=(hc == HPC - 1))
                    bo = bopool.tile([128, 512], F32, tag="bo")
                    if dsp % 2 == 0:
                        nc.scalar.copy(bo[:], pd[:])
                    else:
                        nc.vector.tensor_copy(bo[:], pd[:])
                    nc.sync.dma_start(
                        outp[tt * 128:(tt + 1) * 128, dsp * 512:(dsp + 1) * 512],
                        bo[:])
    nc.compile()
    return nc


def _rope_tables(position_ids_b):
    pos = np.asarray(position_ids_b, dtype=np.float64)
    inv_freq = 1.0 / (BASE ** (np.arange(0, ROT, 2, dtype=np.float64) / ROT))
    freqs = np.outer(pos, inv_freq)                       # [S, 16]
    emb = np.concatenate([freqs, freqs], axis=-1)         # [S, 32]
    return (np.cos(emb).T.astype(NPBF).copy(),
            np.sin(emb).T.astype(NPBF).copy())


def kernel(hidden_states, position_ids, W_qkv, b_qkv, W_dense, b_dense):
    global _NC
    if _NC is None:
        _NC = _build()
    nc = _NC

    hidden_states = np.asarray(hidden_states, dtype=np.float32)
    W_qkv = np.asarray(W_qkv, dtype=np.float32)
    b_qkv = np.asarray(b_qkv, dtype=np.float32)
    W_dense = np.asarray(W_dense, dtype=np.float32)
    b_dense = np.asarray(b_dense, dtype=np.float32)

    rotm = np.zeros((ROT, ROT), np.float32)
    half = ROT // 2
    for i in range(half):
        rotm[i + half, i] = -1.0
        rotm[i, i + half] = 1.0
    ones = np.ones((128, 128), np.float32)

    hTs = [np.ascontiguousarray(hidden_states[b].T).astype(NPBF) for b in range(B)]
    tabs_per_b = [_rope_tables(np.asarray(position_ids)[b]) for b in range(B)]

    in_maps = []
    for c in range(NCORES):
        b, g = divmod(c, CPB)
        # per-head column slices of W_qkv (NeoX fused layout: head-major,
        # [q(128) k(128) v(128)] per head)
        wqk = np.empty((D, 8 * 128), np.float32)
        wv = np.empty((D, HPC * 128), np.float32)
        bqk_host = np.empty((128, 8), np.float32)
        bv_host = np.empty((128, HPC * 128), np.float32)
        for hp in range(HPC):
            c0 = (g * HPC + hp) * 3 * HS
            wqk[:, (2 * hp) * 128:(2 * hp + 1) * 128] = \
                W_qkv[:, c0:c0 + HS] * SM_SCALE
            wqk[:, (2 * hp + 1) * 128:(2 * hp + 2) * 128] = \
                W_qkv[:, c0 + HS:c0 + 2 * HS]
            wv[:, hp * 128:(hp + 1) * 128] = W_qkv[:, c0 + 2 * HS:c0 + 3 * HS]
            bqk_host[:, 2 * hp] = b_qkv[c0:c0 + HS] * SM_SCALE
            bqk_host[:, 2 * hp + 1] = b_qkv[c0 + HS:c0 + 2 * HS]
            bv_host[:, hp * 128:(hp + 1) * 128] = \
                b_qkv[c0 + 2 * HS:c0 + 3 * HS][None, :]
        cosT, sinT = tabs_per_b[b]
        in_maps.append({
            "h16d": hTs[b],
            "wqk16": wqk.astype(NPBF),
            "wv16": wv.astype(NPBF),
            "wd16": np.ascontiguousarray(
                W_dense[g * HPC * HS:(g + 1) * HPC * HS, :]).astype(NPBF),
            "tabc16": cosT,
            "tabs16": sinT,
            "rotm16": rotm.astype(NPBF),
            "ones16": ones.astype(NPBF),
            "bqk": bqk_host,
            "bv": bv_host,
        })

    res = bass_utils.run_bass_kernel_spmd(
        nc, in_maps, core_ids=list(range(NCORES)), trace=TRACE)
    LAST_RESULT[0] = res

    out = np.empty((B, S, D), np.float32)
    for b in range(B):
        acc = np.zeros((S, D), np.float64)
        for g in range(CPB):
            acc += res.results[b * CPB + g]["outp"]
        out[b] = (acc + b_dense).astype(np.float32)
    return out


# revision 13
# speedup vs baseline: 1.1652x; 1.0177x over previous
"""Trainium2 Bass kernel for NeoX-style attention block (B=2, S=2048, D=2048,
H=16, HS=128, partial RoPE rot=32, no mask) sharded over 8 NeuronCores.

Sharding: core c handles batch b = c//4 and head group g = c%4 (4 heads).
Tensor-parallel over heads: W_qkv column-sliced, W_dense row-sliced; each core
produces a partial [S, D] output; host sums 4 partials per batch + bias.

All matmul operands are bf16 (f32 PSUM accumulation): identical PE rate to
f32r at N>=256 but halves DMA/SBUF, enables Fast Weight Load, and doubles DVE
throughput for 16-bit elementwise ops.  Single QKV pass (weights resident).

Per-core dataflow:
  q,k produced transposed (qkT[hs, tok]) via lhsT = W chunks, rhs = hT chunks;
  V produced NATURAL ([tok, hs]) via the swapped matmul lhsT = hT chunk,
  rhs = Wv (kills the per-head PE transposes of V).  Partial RoPE (first 32
  rows of each q/k chunk) via a rotate-half matmul + 3 DVE ops per window.
  Scores^T chunks S^T[k, q] = K^T_chunk.T @ Q^T (512-wide), exp on ACT
  (bf16 out), AV accum O^T = V_chunk.T @ E plus a ones-matmul accumulating
  softmax denominators; normalization = reciprocal_approx_fast + multiply.
  Dense partial uses lhsT = O^T chunks, rhs = W_dense row-slice.
"""
import sys

sys.path.insert(0, "/opt/trn_rl_repo")

import numpy as np
import ml_dtypes
from contextlib import ExitStack

import concourse.bass as bass  # noqa: F401  (registers engine types)
import concourse.tile as tile
from concourse import bacc, mybir
from concourse import bass_utils

F32 = mybir.dt.float32
BF16 = mybir.dt.bfloat16
NPBF = ml_dtypes.bfloat16
MUL = mybir.AluOpType.mult
ADD = mybir.AluOpType.add

B, S, D = 2, 2048, 2048
H, HS, ROT = 16, 128, 32
BASE = 10000.0
SM_SCALE = 1.0 / float(np.sqrt(HS))

HPC = 4            # heads per core
CPB = 4            # cores per batch
NCORES = 8
KC = D // 128      # 16 contraction chunks
NW = 4             # token windows of 512

_NC = None
TRACE = False
LAST_RESULT = [None]


def _build():
    nc = bacc.Bacc("TRN2", target_bir_lowering=False, debug=False)
    # all big inputs are host-prepared in the exact SBUF image layout so each
    # DMA is contiguous per partition (128 fat descriptors, not 2048 thin)
    h16d = nc.dram_tensor("h16d", [128, NW * KC * 512], BF16,
                          kind="ExternalInput").ap()
    wqk16 = nc.dram_tensor("wqk16", [128, 8 * KC * 128], BF16,
                           kind="ExternalInput").ap()
    wv16 = nc.dram_tensor("wv16", [128, KC * 512], BF16,
                          kind="ExternalInput").ap()
    wd16 = nc.dram_tensor("wd16", [128, HPC * D], BF16,
                          kind="ExternalInput").ap()
    tabc16 = nc.dram_tensor("tabc16", [ROT, S], BF16, kind="ExternalInput").ap()
    tabs16 = nc.dram_tensor("tabs16", [ROT, S], BF16, kind="ExternalInput").ap()
    rotm16 = nc.dram_tensor("rotm16", [ROT, ROT], BF16, kind="ExternalInput").ap()
    ones16 = nc.dram_tensor("ones16", [128, 128], BF16, kind="ExternalInput").ap()
    bqk = nc.dram_tensor("bqk", [128, 8], F32, kind="ExternalInput").ap()
    bv = nc.dram_tensor("bv", [128, HPC * 128], F32, kind="ExternalInput").ap()
    outp = nc.dram_tensor("outp", [S, D], F32, kind="ExternalOutput").ap()

    with tile.TileContext(nc) as tc:
        with ExitStack() as ctx:
            glob = ctx.enter_context(tc.tile_pool(name="glob", bufs=1))
            hpool = ctx.enter_context(tc.tile_pool(name="hpool", bufs=2))
            epool = ctx.enter_context(tc.tile_pool(name="epool", bufs=3))
            bopool = ctx.enter_context(tc.tile_pool(name="bopool", bufs=4))
            ps = ctx.enter_context(tc.tile_pool(name="ps", bufs=1, space="PSUM"))

            # ---- resident weights / tables ----
            # DMA issue order is critical for startup latency: first QKV
            # weight chunk + first token window lead; dense weights trail.
            wqk_sb = glob.tile([128, 8 * KC * 128], BF16, tag="wqk")

            def load_wqk(m):
                nc.sync.dma_start(
                    wqk_sb[:, m * 2048:(m + 1) * 2048],
                    wqk16[:, m * 2048:(m + 1) * 2048])

            load_wqk(0)

            def load_ht(n, split=1):
                ht = hpool.tile([128, KC * 512], BF16, tag="ht",
                                name=f"ht{n}")
                w = KC * 512 // split
                for k4 in range(split):
                    nc.sync.dma_start(
                        ht[:, k4 * w:(k4 + 1) * w],
                        h16d[:, n * KC * 512 + k4 * w:
                             n * KC * 512 + (k4 + 1) * w])
                return ht

            ht0 = load_ht(0, split=4)
            load_wqk(1)
            tabc_sb = glob.tile([ROT, S], BF16, tag="tabc")
            nc.sync.dma_start(tabc_sb[:], tabc16)
            tabs_sb = glob.tile([ROT, S], BF16, tag="tabs")
            nc.sync.dma_start(tabs_sb[:], tabs16)
            rot_sb = glob.tile([ROT, ROT], BF16, tag="rotm")
            nc.sync.dma_start(rot_sb[:], rotm16)
            bqk_sb = glob.tile([128, 8], F32, tag="bqk")
            nc.sync.dma_start(bqk_sb[:], bqk)
            for m in range(2, 8):
                load_wqk(m)
            wv_sb = glob.tile([128, KC * 512], BF16, tag="wv")
            nc.sync.dma_start(wv_sb[:], wv16)
            bv_sb = glob.tile([128, HPC * 128], F32, tag="bv")
            nc.sync.dma_start(bv_sb[:], bv)
            ones_sb = glob.tile([128, 128], BF16, tag="ones")
            nc.sync.dma_start(ones_sb[:], ones16)

            # persistent activations
            qkT = glob.tile([128, 8 * S], BF16, tag="qkT")   # [hs, m*S + tok]
            vn = glob.tile([128, HPC * S], BF16, tag="vn")   # [ktok, h*S+kc*128+hs]
            oT = glob.tile([128, HPC * S], BF16, tag="oT")   # [hs, h*S + tok]
            wd_sb = glob.tile([128, HPC * D], BF16, tag="wd")

            def rope(m, n):
                # rotate first ROT dims of q/k chunk m for token window n.
                # Issued one m-chunk late so the rot matmul's dependency (the
                # DVE eviction of chunk m) is already done when PE reaches it.
                sl = slice(m * S + n * 512, m * S + (n + 1) * 512)
                wsl = slice(n * 512, (n + 1) * 512)
                pr = ps.tile([128, 512], F32, tag="v5", bufs=4, name=f"pr{m}_{n}")
                nc.tensor.matmul(pr[0:ROT, :], rot_sb[:, :], qkT[0:ROT, sl],
                                 start=True, stop=True)
                nc.vector.tensor_tensor(pr[0:ROT, :], pr[0:ROT, :],
                                        tabs_sb[:, wsl], op=MUL)
                nc.vector.tensor_tensor(qkT[0:ROT, sl], qkT[0:ROT, sl],
                                        tabc_sb[:, wsl], op=MUL)
                nc.vector.tensor_tensor(qkT[0:ROT, sl], qkT[0:ROT, sl],
                                        pr[0:ROT, :], op=ADD)

            # ---- QKV projection (single pass, all 4 heads) ----
            for n in range(NW):
                ht = ht0 if n == 0 else load_ht(n)
                for m in range(8):
                    pq = ps.tile([128, 512], F32, tag="v5", bufs=4,
                                 name=f"pq{m}_{n}")
                    for kc in range(KC):
                        nc.tensor.matmul(
                            pq[:],
                            wqk_sb[:, m * 2048 + kc * 128:m * 2048 + (kc + 1) * 128],
                            ht[:, kc * 512:(kc + 1) * 512],
                            start=(kc == 0), stop=(kc == KC - 1))
                    nc.vector.tensor_scalar_add(
                        qkT[:, m * S + n * 512:m * S + (n + 1) * 512],
                        pq[:], bqk_sb[:, m:m + 1])
                    if m > 0:
                        rope(m - 1, n)
                for t4 in range(4):
                    pv = ps.tile([128, 512], F32, tag="v5", bufs=4,
                                 name=f"pv{t4}_{n}")
                    for kc in range(KC):
                        nc.tensor.matmul(
                            pv[:],
                            ht[:, kc * 512 + t4 * 128:kc * 512 + (t4 + 1) * 128],
                            wv_sb[:, kc * 512:(kc + 1) * 512],
                            start=(kc == 0), stop=(kc == KC - 1))
                    tc4 = n * 4 + t4     # global 128-token chunk index
                    nc.vector.tensor_tensor(
                        vn.rearrange("p (h t) -> p h t", h=HPC)[
                            :, :, tc4 * 128:(tc4 + 1) * 128],
                        pv[:].rearrange("p (h t) -> p h t", h=HPC),
                        bv_sb.rearrange("p (h t) -> p h t", h=HPC),
                        op=ADD)
                    if n == NW - 1 and t4 == 0:
                        rope(7, n)
                if n < NW - 1:
                    rope(7, n)

            # ---- attention, software-pipelined one exp behind ----
            prev = [None]

            def consume(h, qs, k2, po, pm, e):
                for j in range(2):
                    kc = 2 * k2 + j
                    nc.tensor.matmul(po, vn[:, h * S + kc * 128:h * S + (kc + 1) * 128],
                                     e[:, j * 512:(j + 1) * 512],
                                     start=(kc == 0), stop=(kc == KC - 1))
                for j in range(2):
                    kc = 2 * k2 + j
                    nc.tensor.matmul(pm, ones_sb[:], e[:, j * 512:(j + 1) * 512],
                                     start=(kc == 0), stop=(kc == KC - 1))
                if k2 == KC // 2 - 1:
                    rc = epool.tile([128, 512], F32, tag="rc", bufs=2,
                                    name=f"rc{h}_{qs}")
                    nc.vector.reciprocal_approx_fast(rc[:], pm)
                    nc.vector.tensor_tensor(
                        oT[:, h * S + qs * 512:h * S + (qs + 1) * 512],
                        po, rc[:], op=MUL)

            def attention_head(h):
                qoff, koff = (2 * h) * S, (2 * h + 1) * S
                for qs in range(4):
                    po = ps.tile([128, 512], F32, tag="v5", bufs=4,
                                 name=f"po{h}_{qs}")
                    pm = ps.tile([128, 512], F32, tag="v5", bufs=4,
                                 name=f"pm{h}_{qs}")
                    for k2 in range(KC // 2):
                        pS = ps.tile([128, 1024], F32, tag="pS", bufs=2,
                                     name=f"pS{h}_{qs}_{k2}")
                        for j in range(2):
                            kc = 2 * k2 + j
                            nc.tensor.matmul(
                                pS[:, j * 512:(j + 1) * 512],
                                qkT[:, koff + kc * 128:koff + (kc + 1) * 128],
                                qkT[:, qoff + qs * 512:qoff + (qs + 1) * 512],
                                start=True, stop=True)
                        e = epool.tile([128, 1024], BF16, tag="e")
                        nc.scalar.activation(e[:], pS[:],
                                             mybir.ActivationFunctionType.Exp)
                        if prev[0] is not None:
                            consume(*prev[0])
                        prev[0] = (h, qs, k2, po, pm, e)
                if h == 0:
                    # dense weights: issued here so the transfer overlaps
                    # attention; needed only at the dense stage
                    nc.sync.dma_start(wd_sb[:], wd16)

            for h in range(HPC):
                attention_head(h)
            consume(*prev[0])

            # ---- dense partial ----
            for tt in range(S // 128):
                for d2 in range(2):
                    bo = bopool.tile([128, 1024], F32, tag="bo",
                                     name=f"bo{tt}_{d2}")
                    for j in range(2):
                        dsp = 2 * d2 + j
                        pd = ps.tile([128, 512], F32, tag="v5", bufs=4,
                                     name=f"pd{tt}_{dsp}")
                        for hc in range(HPC):
                            nc.tensor.matmul(
                                pd[:],
                                oT[:, hc * S + tt * 128:hc * S + (tt + 1) * 128],
                                wd_sb[:, hc * D + dsp * 512:hc * D + (dsp + 1) * 512],
                                start=(hc == 0), stop=(hc == HPC - 1))
                        if j == 0:
                            nc.scalar.copy(bo[:, 0:512], pd[:])
                        else:
                            nc.vector.tensor_copy(bo[:, 512:1024], pd[:])
                    nc.sync.dma_start(
                        outp[tt * 128:(tt + 1) * 128,
                             d2 * 1024:(d2 + 1) * 1024],
                        bo[:])
    nc.compile()
    return nc


def _rope_tables(position_ids_b):
    pos = np.asarray(position_ids_b, dtype=np.float64)
    inv_freq = 1.0 / (BASE ** (np.arange(0, ROT, 2, dtype=np.float64) / ROT))
    freqs = np.outer(pos, inv_freq)                       # [S, 16]
    emb = np.concatenate([freqs, freqs], axis=-1)         # [S, 32]
    return (np.cos(emb).T.astype(NPBF).copy(),
            np.sin(emb).T.astype(NPBF).copy())


def kernel(hidden_states, position_ids, W_qkv, b_qkv, W_dense, b_dense):
    global _NC
    if _NC is None:
        _NC = _build()
    nc = _NC

    hidden_states = np.asarray(hidden_states, dtype=np.float32)
    W_qkv = np.asarray(W_qkv, dtype=np.float32)
    b_qkv = np.asarray(b_qkv, dtype=np.float32)
    W_dense = np.asarray(W_dense, dtype=np.float32)
    b_dense = np.asarray(b_dense, dtype=np.float32)

    rotm = np.zeros((ROT, ROT), np.float32)
    half = ROT // 2
    for i in range(half):
        rotm[i + half, i] = -1.0
        rotm[i, i + half] = 1.0
    ones = np.ones((128, 128), np.float32)

    # hidden image: h_img[p, n*8192 + kc*512 + t] = hidden[b].T[kc*128+p, n*512+t]
    hTs = []
    for b in range(B):
        hT = hidden_states[b].T.reshape(KC, 128, NW, 512)
        hTs.append(np.ascontiguousarray(
            hT.transpose(1, 2, 0, 3).reshape(128, NW * KC * 512)).astype(NPBF))
    tabs_per_b = [_rope_tables(np.asarray(position_ids)[b]) for b in range(B)]

    in_maps = []
    for c in range(NCORES):
        b, g = divmod(c, CPB)
        # per-head column slices of W_qkv (NeoX fused layout: head-major,
        # [q(128) k(128) v(128)] per head)
        wqk = np.empty((D, 8 * 128), np.float32)
        wv = np.empty((D, HPC * 128), np.float32)
        bqk_host = np.empty((128, 8), np.float32)
        bv_host = np.empty((128, HPC * 128), np.float32)
        for hp in range(HPC):
            c0 = (g * HPC + hp) * 3 * HS
            wqk[:, (2 * hp) * 128:(2 * hp + 1) * 128] = \
                W_qkv[:, c0:c0 + HS] * SM_SCALE
            wqk[:, (2 * hp + 1) * 128:(2 * hp + 2) * 128] = \
                W_qkv[:, c0 + HS:c0 + 2 * HS]
            wv[:, hp * 128:(hp + 1) * 128] = W_qkv[:, c0 + 2 * HS:c0 + 3 * HS]
            bqk_host[:, 2 * hp] = b_qkv[c0:c0 + HS] * SM_SCALE
            bqk_host[:, 2 * hp + 1] = b_qkv[c0 + HS:c0 + 2 * HS]
            bv_host[:, hp * 128:(hp + 1) * 128] = \
                b_qkv[c0 + 2 * HS:c0 + 3 * HS][None, :]
        # SBUF-image layouts: [p, m*2048 + kc*128 + c], [p, kc*512 + c],
        # [p, hc*2048 + d]
        wqk_img = wqk.reshape(KC, 128, 8 * 128).transpose(1, 0, 2)  # p, kc, mc
        wqk_img = np.ascontiguousarray(
            wqk_img.reshape(128, KC, 8, 128).transpose(0, 2, 1, 3)
            .reshape(128, 8 * KC * 128))
        wv_img = np.ascontiguousarray(
            wv.reshape(KC, 128, HPC * 128).transpose(1, 0, 2)
            .reshape(128, KC * HPC * 128))
        wd_img = np.ascontiguousarray(
            W_dense[g * HPC * HS:(g + 1) * HPC * HS, :]
            .reshape(HPC, 128, D).transpose(1, 0, 2).reshape(128, HPC * D))
        cosT, sinT = tabs_per_b[b]
        in_maps.append({
            "h16d": hTs[b],
            "wqk16": wqk_img.astype(NPBF),
            "wv16": wv_img.astype(NPBF),
            "wd16": wd_img.astype(NPBF),
            "tabc16": cosT,
            "tabs16": sinT,
            "rotm16": rotm.astype(NPBF),
            "ones16": ones.astype(NPBF),
            "bqk": bqk_host,
            "bv": bv_host,
        })

    res = bass_utils.run_bass_kernel_spmd(
        nc, in_maps, core_ids=list(range(NCORES)), trace=TRACE)
    LAST_RESULT[0] = res

    out = np.empty((B, S, D), np.float32)
    for b in range(B):
        acc = np.zeros((S, D), np.float64)
        for g in range(CPB):
            acc += res.results[b * CPB + g]["outp"]
        out[b] = (acc + b_dense).astype(np.float32)
    return out


# revision 21
# speedup vs baseline: 1.1817x; 1.0141x over previous
"""Trainium2 Bass kernel for NeoX-style attention block (B=2, S=2048, D=2048,
H=16, HS=128, partial RoPE rot=32, no mask) sharded over 8 NeuronCores.

Sharding: core c handles batch b = c//4 and head group g = c%4 (4 heads).
Tensor-parallel over heads: W_qkv column-sliced, W_dense row-sliced; each core
produces a partial [S, D] output; host sums 4 partials per batch + bias.

All matmul operands are bf16 (f32 PSUM accumulation): identical PE rate to
f32r at N>=256 but halves DMA/SBUF, enables Fast Weight Load, and doubles DVE
throughput for 16-bit elementwise ops.  Single QKV pass (weights resident).

Per-core dataflow:
  q,k produced transposed (qkT[hs, tok]) via lhsT = W chunks, rhs = hT chunks;
  V produced NATURAL ([tok, hs]) via the swapped matmul lhsT = hT chunk,
  rhs = Wv (kills the per-head PE transposes of V).  Partial RoPE (first 32
  rows of each q/k chunk) via a rotate-half matmul + 3 DVE ops per window.
  Scores^T chunks S^T[k, q] = K^T_chunk.T @ Q^T (512-wide), exp on ACT
  (bf16 out), AV accum O^T = V_chunk.T @ E plus a ones-matmul accumulating
  softmax denominators; normalization = reciprocal_approx_fast + multiply.
  Dense partial uses lhsT = O^T chunks, rhs = W_dense row-slice.
"""
import sys

sys.path.insert(0, "/opt/trn_rl_repo")

import numpy as np
import ml_dtypes
from contextlib import ExitStack

import concourse.bass as bass  # noqa: F401  (registers engine types)
import concourse.tile as tile
from concourse import bacc, mybir
from concourse import bass_utils

F32 = mybir.dt.float32
BF16 = mybir.dt.bfloat16
NPBF = ml_dtypes.bfloat16
MUL = mybir.AluOpType.mult
ADD = mybir.AluOpType.add

B, S, D = 2, 2048, 2048
H, HS, ROT = 16, 128, 32
BASE = 10000.0
SM_SCALE = 1.0 / float(np.sqrt(HS))

HPC = 4            # heads per core
CPB = 4            # cores per batch
NCORES = 8
KC = D // 128      # 16 contraction chunks
NW = 4             # token windows of 512

_NC = None
TRACE = False
LAST_RESULT = [None]


def _build():
    nc = bacc.Bacc("TRN2", target_bir_lowering=False, debug=False)
    # all big inputs are host-prepared in the exact SBUF image layout so each
    # DMA is contiguous per partition (128 fat descriptors, not 2048 thin)
    h16d = nc.dram_tensor("h16d", [128, NW * KC * 512], BF16,
                          kind="ExternalInput").ap()
    wqk16 = nc.dram_tensor("wqk16", [128, 8 * KC * 128], BF16,
                           kind="ExternalInput").ap()
    wv16 = nc.dram_tensor("wv16", [128, KC * 512], BF16,
                          kind="ExternalInput").ap()
    wd16 = nc.dram_tensor("wd16", [128, HPC * D], BF16,
                          kind="ExternalInput").ap()
    tabc16 = nc.dram_tensor("tabc16", [ROT, S], BF16, kind="ExternalInput").ap()
    tabs16 = nc.dram_tensor("tabs16", [ROT, S], BF16, kind="ExternalInput").ap()
    rotm16 = nc.dram_tensor("rotm16", [ROT, ROT], BF16, kind="ExternalInput").ap()
    ones16 = nc.dram_tensor("ones16", [128, 128], BF16, kind="ExternalInput").ap()
    bqk = nc.dram_tensor("bqk", [128, 8], F32, kind="ExternalInput").ap()
    bv = nc.dram_tensor("bv", [128, HPC * 128], F32, kind="ExternalInput").ap()
    outp = nc.dram_tensor("outp", [S, D], F32, kind="ExternalOutput").ap()

    with tile.TileContext(nc) as tc:
        with ExitStack() as ctx:
            glob = ctx.enter_context(tc.tile_pool(name="glob", bufs=1))
            hpool = ctx.enter_context(tc.tile_pool(name="hpool", bufs=2))
            epool = ctx.enter_context(tc.tile_pool(name="epool", bufs=3))
            bopool = ctx.enter_context(tc.tile_pool(name="bopool", bufs=4))
            ps = ctx.enter_context(tc.tile_pool(name="ps", bufs=1, space="PSUM"))

            # ---- resident weights / tables ----
            # DMA issue order is critical for startup latency: first QKV
            # weight chunk + first token window lead; dense weights trail.
            wqk_sb = glob.tile([128, 8 * KC * 128], BF16, tag="wqk")

            def load_wqk(m):
                nc.sync.dma_start(
                    wqk_sb[:, m * 2048:(m + 1) * 2048],
                    wqk16[:, m * 2048:(m + 1) * 2048])

            load_wqk(0)

            def load_ht(n, engines=None):
                ht = hpool.tile([128, KC * 512], BF16, tag="ht",
                                name=f"ht{n}")
                engines = engines or [nc.sync]
                w = KC * 512 // len(engines)
                for k4, eng in enumerate(engines):
                    eng.dma_start(
                        ht[:, k4 * w:(k4 + 1) * w],
                        h16d[:, n * KC * 512 + k4 * w:
                             n * KC * 512 + (k4 + 1) * w])
                return ht

            # first window: issue from otherwise-idle engine queues in
            # parallel with wqk on sync, to cut the serialized-issue head
            ht0 = load_ht(0, engines=[nc.scalar, nc.scalar])
            load_wqk(1)
            tabc_sb = glob.tile([ROT, S], BF16, tag="tabc")
            nc.sync.dma_start(tabc_sb[:], tabc16)
            tabs_sb = glob.tile([ROT, S], BF16, tag="tabs")
            nc.sync.dma_start(tabs_sb[:], tabs16)
            rot_sb = glob.tile([ROT, ROT], BF16, tag="rotm")
            nc.sync.dma_start(rot_sb[:], rotm16)
            bqk_sb = glob.tile([128, 8], F32, tag="bqk")
            nc.sync.dma_start(bqk_sb[:], bqk)
            for m in range(2, 8):
                load_wqk(m)
            wv_sb = glob.tile([128, KC * 512], BF16, tag="wv")
            nc.sync.dma_start(wv_sb[:], wv16)
            bv_sb = glob.tile([128, HPC * 128], F32, tag="bv")
            nc.sync.dma_start(bv_sb[:], bv)
            ones_sb = glob.tile([128, 128], BF16, tag="ones")
            nc.sync.dma_start(ones_sb[:], ones16)

            # persistent activations
            qkT = glob.tile([128, 8 * S], BF16, tag="qkT")   # [hs, m*S + tok]
            vn = glob.tile([128, HPC * S], BF16, tag="vn")   # [ktok, h*S+kc*128+hs]
            oT = glob.tile([128, HPC * S], BF16, tag="oT")   # [hs, h*S + tok]
            wd_sb = glob.tile([128, HPC * D], BF16, tag="wd")

            def rope(m, n):
                # rotate first ROT dims of q/k chunk m for token window n.
                # Issued one m-chunk late so the rot matmul's dependency (the
                # DVE eviction of chunk m) is already done when PE reaches it.
                sl = slice(m * S + n * 512, m * S + (n + 1) * 512)
                wsl = slice(n * 512, (n + 1) * 512)
                pr = ps.tile([128, 512], F32, tag="v5", bufs=4, name=f"pr{m}_{n}")
                nc.tensor.matmul(pr[0:ROT, :], rot_sb[:, :], qkT[0:ROT, sl],
                                 start=True, stop=True)
                nc.vector.tensor_tensor(pr[0:ROT, :], pr[0:ROT, :],
                                        tabs_sb[:, wsl], op=MUL)
                nc.vector.tensor_tensor(qkT[0:ROT, sl], qkT[0:ROT, sl],
                                        tabc_sb[:, wsl], op=MUL)
                nc.vector.tensor_tensor(qkT[0:ROT, sl], qkT[0:ROT, sl],
                                        pr[0:ROT, :], op=ADD)

            # ---- QKV projection (single pass, all 4 heads) ----
            # rope matmuls are batched (not interleaved between full-K
            # chains) to minimize K-size transitions on the PE
            for n in range(NW):
                ht = ht0 if n == 0 else load_ht(n)
                for m in range(8):
                    pq = ps.tile([128, 512], F32, tag="v5", bufs=4,
                                 name=f"pq{m}_{n}")
                    for kc in range(KC):
                        nc.tensor.matmul(
                            pq[:],
                            wqk_sb[:, m * 2048 + kc * 128:m * 2048 + (kc + 1) * 128],
                            ht[:, kc * 512:(kc + 1) * 512],
                            start=(kc == 0), stop=(kc == KC - 1))
                    nc.vector.tensor_scalar_add(
                        qkT[:, m * S + n * 512:m * S + (n + 1) * 512],
                        pq[:], bqk_sb[:, m:m + 1])
                    if m > 0:
                        rope(m - 1, n)
                for t4 in range(4):
                    pv = ps.tile([128, 512], F32, tag="v5", bufs=4,
                                 name=f"pv{t4}_{n}")
                    for kc in range(KC):
                        nc.tensor.matmul(
                            pv[:],
                            ht[:, kc * 512 + t4 * 128:kc * 512 + (t4 + 1) * 128],
                            wv_sb[:, kc * 512:(kc + 1) * 512],
                            start=(kc == 0), stop=(kc == KC - 1))
                    tc4 = n * 4 + t4     # global 128-token chunk index
                    nc.vector.tensor_tensor(
                        vn.rearrange("p (h t) -> p h t", h=HPC)[
                            :, :, tc4 * 128:(tc4 + 1) * 128],
                        pv[:].rearrange("p (h t) -> p h t", h=HPC),
                        bv_sb.rearrange("p (h t) -> p h t", h=HPC),
                        op=ADD)
                    if t4 == 0:
                        rope(7, n)

            # ---- attention, software-pipelined TWO exps behind (the AV
            # matmuls of round k2 run while exp of k2+1 is in flight, so the
            # PE never waits on ACT latency) ----
            pending = []

            def consume(h, qs, k2, po, pm, e):
                for j in range(2):
                    kc = 2 * k2 + j
                    nc.tensor.matmul(po, vn[:, h * S + kc * 128:h * S + (kc + 1) * 128],
                                     e[:, j * 512:(j + 1) * 512],
                                     start=(kc == 0), stop=(kc == KC - 1))
                for j in range(2):
                    kc = 2 * k2 + j
                    nc.tensor.matmul(pm, ones_sb[:], e[:, j * 512:(j + 1) * 512],
                                     start=(kc == 0), stop=(kc == KC - 1))
                if k2 == KC // 2 - 1:
                    rc = epool.tile([128, 512], F32, tag="rc", bufs=2,
                                    name=f"rc{h}_{qs}")
                    nc.vector.reciprocal_approx_fast(rc[:], pm)
                    nc.vector.tensor_tensor(
                        oT[:, h * S + qs * 512:h * S + (qs + 1) * 512],
                        po, rc[:], op=MUL)

            def attention_head(h):
                qoff, koff = (2 * h) * S, (2 * h + 1) * S
                for qs in range(4):
                    po = ps.tile([128, 512], F32, tag="v5", bufs=4,
                                 name=f"po{h}_{qs}")
                    pm = ps.tile([128, 512], F32, tag="v5", bufs=4,
                                 name=f"pm{h}_{qs}")
                    for k2 in range(KC // 2):
                        pS = ps.tile([128, 1024], F32, tag="pS", bufs=2,
                                     name=f"pS{h}_{qs}_{k2}")
                        for j in range(2):
                            kc = 2 * k2 + j
                            nc.tensor.matmul(
                                pS[:, j * 512:(j + 1) * 512],
                                qkT[:, koff + kc * 128:koff + (kc + 1) * 128],
                                qkT[:, qoff + qs * 512:qoff + (qs + 1) * 512],
                                start=True, stop=True)
                        e = epool.tile([128, 1024], BF16, tag="e", bufs=4)
                        nc.scalar.activation(e[:], pS[:],
                                             mybir.ActivationFunctionType.Exp)
                        if len(pending) >= 2:
                            consume(*pending.pop(0))
                        pending.append((h, qs, k2, po, pm, e))
                if h == 0:
                    # dense weights: issued here so the transfer overlaps
                    # attention; needed only at the dense stage
                    nc.sync.dma_start(wd_sb[:], wd16)

            for h in range(HPC):
                attention_head(h)
            while pending:
                consume(*pending.pop(0))

            # ---- dense partial ----
            for tt in range(S // 128):
                for d2 in range(2):
                    bo = bopool.tile([128, 1024], F32, tag="bo",
                                     name=f"bo{tt}_{d2}")
                    for j in range(2):
                        dsp = 2 * d2 + j
                        pd = ps.tile([128, 512], F32, tag="v5", bufs=4,
                                     name=f"pd{tt}_{dsp}")
                        for hc in range(HPC):
                            nc.tensor.matmul(
                                pd[:],
                                oT[:, hc * S + tt * 128:hc * S + (tt + 1) * 128],
                                wd_sb[:, hc * D + dsp * 512:hc * D + (dsp + 1) * 512],
                                start=(hc == 0), stop=(hc == HPC - 1))
                        if j == 0:
                            nc.scalar.copy(bo[:, 0:512], pd[:])
                        else:
                            nc.vector.tensor_copy(bo[:, 512:1024], pd[:])
                    nc.sync.dma_start(
                        outp[tt * 128:(tt + 1) * 128,
                             d2 * 1024:(d2 + 1) * 1024],
                        bo[:])
    nc.compile()
    return nc


def _rope_tables(position_ids_b):
    pos = np.asarray(position_ids_b, dtype=np.float64)
    inv_freq = 1.0 / (BASE ** (np.arange(0, ROT, 2, dtype=np.float64) / ROT))
    freqs = np.outer(pos, inv_freq)                       # [S, 16]
    emb = np.concatenate([freqs, freqs], axis=-1)         # [S, 32]
    return (np.cos(emb).T.astype(NPBF).copy(),
            np.sin(emb).T.astype(NPBF).copy())


def kernel(hidden_states, position_ids, W_qkv, b_qkv, W_dense, b_dense):
    global _NC
    if _NC is None:
        _NC = _build()
    nc = _NC

    hidden_states = np.asarray(hidden_states, dtype=np.float32)
    W_qkv = np.asarray(W_qkv, dtype=np.float32)
    b_qkv = np.asarray(b_qkv, dtype=np.float32)
    W_dense = np.asarray(W_dense, dtype=np.float32)
    b_dense = np.asarray(b_dense, dtype=np.float32)

    rotm = np.zeros((ROT, ROT), np.float32)
    half = ROT // 2
    for i in range(half):
        rotm[i + half, i] = -1.0
        rotm[i, i + half] = 1.0
    ones = np.ones((128, 128), np.float32)

    # hidden image: h_img[p, n*8192 + kc*512 + t] = hidden[b].T[kc*128+p, n*512+t]
    hTs = []
    for b in range(B):
        hT = hidden_states[b].T.reshape(KC, 128, NW, 512)
        hTs.append(np.ascontiguousarray(
            hT.transpose(1, 2, 0, 3).reshape(128, NW * KC * 512)).astype(NPBF))
    tabs_per_b = [_rope_tables(np.asarray(position_ids)[b]) for b in range(B)]

    in_maps = []
    for c in range(NCORES):
        b, g = divmod(c, CPB)
        # per-head column slices of W_qkv (NeoX fused layout: head-major,
        # [q(128) k(128) v(128)] per head)
        wqk = np.empty((D, 8 * 128), np.float32)
        wv = np.empty((D, HPC * 128), np.float32)
        bqk_host = np.empty((128, 8), np.float32)
        bv_host = np.empty((128, HPC * 128), np.float32)
        for hp in range(HPC):
            c0 = (g * HPC + hp) * 3 * HS
            wqk[:, (2 * hp) * 128:(2 * hp + 1) * 128] = \
                W_qkv[:, c0:c0 + HS] * SM_SCALE
            wqk[:, (2 * hp + 1) * 128:(2 * hp + 2) * 128] = \
                W_qkv[:, c0 + HS:c0 + 2 * HS]
            wv[:, hp * 128:(hp + 1) * 128] = W_qkv[:, c0 + 2 * HS:c0 + 3 * HS]
            bqk_host[:, 2 * hp] = b_qkv[c0:c0 + HS] * SM_SCALE
            bqk_host[:, 2 * hp + 1] = b_qkv[c0 + HS:c0 + 2 * HS]
            bv_host[:, hp * 128:(hp + 1) * 128] = \
                b_qkv[c0 + 2 * HS:c0 + 3 * HS][None, :]
        # SBUF-image layouts: [p, m*2048 + kc*128 + c], [p, kc*512 + c],
        # [p, hc*2048 + d]
        wqk_img = wqk.reshape(KC, 128, 8 * 128).transpose(1, 0, 2)  # p, kc, mc
        wqk_img = np.ascontiguousarray(
            wqk_img.reshape(128, KC, 8, 128).transpose(0, 2, 1, 3)
            .reshape(128, 8 * KC * 128))
        wv_img = np.ascontiguousarray(
            wv.reshape(KC, 128, HPC * 128).transpose(1, 0, 2)
            .reshape(128, KC * HPC * 128))
        wd_img = np.ascontiguousarray(
            W_dense[g * HPC * HS:(g + 1) * HPC * HS, :]
            .reshape(HPC, 128, D).transpose(1, 0, 2).reshape(128, HPC * D))
        cosT, sinT = tabs_per_b[b]
        in_maps.append({
            "h16d": hTs[b],
            "wqk16": wqk_img.astype(NPBF),
            "wv16": wv_img.astype(NPBF),
            "wd16": wd_img.astype(NPBF),
            "tabc16": cosT,
            "tabs16": sinT,
            "rotm16": rotm.astype(NPBF),
            "ones16": ones.astype(NPBF),
            "bqk": bqk_host,
            "bv": bv_host,
        })

    res = bass_utils.run_bass_kernel_spmd(
        nc, in_maps, core_ids=list(range(NCORES)), trace=TRACE)
    LAST_RESULT[0] = res

    out = np.empty((B, S, D), np.float32)
    for b in range(B):
        acc = np.zeros((S, D), np.float64)
        for g in range(CPB):
            acc += res.results[b * CPB + g]["outp"]
        out[b] = (acc + b_dense).astype(np.float32)
    return out


# revision 26
# speedup vs baseline: 1.1846x; 1.0024x over previous
"""Trainium2 Bass kernel for NeoX-style attention block (B=2, S=2048, D=2048,
H=16, HS=128, partial RoPE rot=32, no mask) sharded over 8 NeuronCores.

Sharding: core c handles batch b = c//4 and head group g = c%4 (4 heads).
Tensor-parallel over heads: W_qkv column-sliced, W_dense row-sliced; each core
produces a partial [S, D] output; host sums 4 partials per batch + bias.

All matmul operands are bf16 (f32 PSUM accumulation): identical PE rate to
f32r at N>=256 but halves DMA/SBUF, enables Fast Weight Load, and doubles DVE
throughput for 16-bit elementwise ops.  Single QKV pass (weights resident).

Per-core dataflow:
  q,k produced transposed (qkT[hs, tok]) via lhsT = W chunks, rhs = hT chunks;
  V produced NATURAL ([tok, hs]) via the swapped matmul lhsT = hT chunk,
  rhs = Wv (kills the per-head PE transposes of V).  Partial RoPE (first 32
  rows of each q/k chunk) via a rotate-half matmul + 3 DVE ops per window.
  Scores^T chunks S^T[k, q] = K^T_chunk.T @ Q^T (512-wide), exp on ACT
  (bf16 out), AV accum O^T = V_chunk.T @ E plus a ones-matmul accumulating
  softmax denominators; normalization = reciprocal_approx_fast + multiply.
  Dense partial uses lhsT = O^T chunks, rhs = W_dense row-slice.
"""
import sys

sys.path.insert(0, "/opt/trn_rl_repo")

import numpy as np
import ml_dtypes
from contextlib import ExitStack

import concourse.bass as bass  # noqa: F401  (registers engine types)
import concourse.tile as tile
from concourse import bacc, mybir
from concourse import bass_utils

F32 = mybir.dt.float32
BF16 = mybir.dt.bfloat16
NPBF = ml_dtypes.bfloat16
MUL = mybir.AluOpType.mult
ADD = mybir.AluOpType.add

B, S, D = 2, 2048, 2048
H, HS, ROT = 16, 128, 32
BASE = 10000.0
SM_SCALE = 1.0 / float(np.sqrt(HS))

HPC = 4            # heads per core
CPB = 4            # cores per batch
NCORES = 8
KC = D // 128      # 16 contraction chunks
NW = 4             # token windows of 512

_NC = None
TRACE = False
LAST_RESULT = [None]


def _build():
    nc = bacc.Bacc("TRN2", target_bir_lowering=False, debug=False)
    # all big inputs are host-prepared in the exact SBUF image layout so each
    # DMA is contiguous per partition (128 fat descriptors, not 2048 thin)
    h16d = nc.dram_tensor("h16d", [128, NW * KC * 512], BF16,
                          kind="ExternalInput").ap()
    wqk16 = nc.dram_tensor("wqk16", [128, 8 * KC * 128], BF16,
                           kind="ExternalInput").ap()
    wv16 = nc.dram_tensor("wv16", [128, KC * 512], BF16,
                          kind="ExternalInput").ap()
    wd16 = nc.dram_tensor("wd16", [128, HPC * D], BF16,
                          kind="ExternalInput").ap()
    tabc16 = nc.dram_tensor("tabc16", [ROT, S], BF16, kind="ExternalInput").ap()
    tabs16 = nc.dram_tensor("tabs16", [ROT, S], BF16, kind="ExternalInput").ap()
    rotm16 = nc.dram_tensor("rotm16", [128, ROT], BF16, kind="ExternalInput").ap()
    ones16 = nc.dram_tensor("ones16", [128, 128], BF16, kind="ExternalInput").ap()
    bqk = nc.dram_tensor("bqk", [128, 8], F32, kind="ExternalInput").ap()
    bv = nc.dram_tensor("bv", [128, HPC * 128], F32, kind="ExternalInput").ap()
    outp = nc.dram_tensor("outp", [S, D], F32, kind="ExternalOutput").ap()

    with tile.TileContext(nc) as tc:
        with ExitStack() as ctx:
            glob = ctx.enter_context(tc.tile_pool(name="glob", bufs=1))
            hpool = ctx.enter_context(tc.tile_pool(name="hpool", bufs=2))
            epool = ctx.enter_context(tc.tile_pool(name="epool", bufs=3))
            bopool = ctx.enter_context(tc.tile_pool(name="bopool", bufs=4))
            ps = ctx.enter_context(tc.tile_pool(name="ps", bufs=1, space="PSUM"))

            # ---- resident weights / tables ----
            # DMA issue order is critical for startup latency: first QKV
            # weight chunk + first token window lead; dense weights trail.
            wqk_sb = glob.tile([128, 8 * KC * 128], BF16, tag="wqk")

            def load_wqk(m):
                nc.sync.dma_start(
                    wqk_sb[:, m * 2048:(m + 1) * 2048],
                    wqk16[:, m * 2048:(m + 1) * 2048])

            load_wqk(0)

            def load_ht(n, engines=None):
                ht = hpool.tile([128, KC * 512], BF16, tag="ht",
                                name=f"ht{n}")
                engines = engines or [nc.sync]
                w = KC * 512 // len(engines)
                for k4, eng in enumerate(engines):
                    eng.dma_start(
                        ht[:, k4 * w:(k4 + 1) * w],
                        h16d[:, n * KC * 512 + k4 * w:
                             n * KC * 512 + (k4 + 1) * w])
                return ht

            # first window: issue from otherwise-idle engine queues in
            # parallel with wqk on sync, to cut the serialized-issue head
            ht0 = load_ht(0, engines=[nc.scalar] * 4)
            load_wqk(1)
            tabc_sb = glob.tile([ROT, S], BF16, tag="tabc")
            nc.sync.dma_start(tabc_sb[:], tabc16)
            tabs_sb = glob.tile([ROT, S], BF16, tag="tabs")
            nc.sync.dma_start(tabs_sb[:], tabs16)
            # rotate-half matrix zero-padded to K=128 so the rope matmul keeps
            # the same contraction size as its neighbors (no HAM transition)
            rot_sb = glob.tile([128, ROT], BF16, tag="rotm")
            nc.sync.dma_start(rot_sb[:], rotm16)
            bqk_sb = glob.tile([128, 8], F32, tag="bqk")
            nc.sync.dma_start(bqk_sb[:], bqk)
            for m in range(2, 8):
                load_wqk(m)
            wv_sb = glob.tile([128, KC * 512], BF16, tag="wv")
            nc.sync.dma_start(wv_sb[:], wv16)
            bv_sb = glob.tile([128, HPC * 128], F32, tag="bv")
            nc.sync.dma_start(bv_sb[:], bv)
            ones_sb = glob.tile([128, 128], BF16, tag="ones")
            nc.sync.dma_start(ones_sb[:], ones16)

            # persistent activations
            qkT = glob.tile([128, 8 * S], BF16, tag="qkT")   # [hs, m*S + tok]
            vn = glob.tile([128, HPC * S], BF16, tag="vn")   # [ktok, h*S+kc*128+hs]
            oT = glob.tile([128, HPC * S], BF16, tag="oT")   # [hs, h*S + tok]
            wd_sb = glob.tile([128, HPC * D], BF16, tag="wd")

            def rope(m, n):
                # rotate first ROT dims of q/k chunk m for token window n.
                # Issued one m-chunk late so the rot matmul's dependency (the
                # DVE eviction of chunk m) is already done when PE reaches it.
                sl = slice(m * S + n * 512, m * S + (n + 1) * 512)
                wsl = slice(n * 512, (n + 1) * 512)
                pr = ps.tile([128, 512], F32, tag="v5", bufs=4, name=f"pr{m}_{n}")
                nc.tensor.matmul(pr[0:ROT, :], rot_sb[:, :],
                                 qkT[:, sl], start=True, stop=True)
                nc.vector.tensor_tensor(pr[0:ROT, :], pr[0:ROT, :],
                                        tabs_sb[:, wsl], op=MUL)
                nc.vector.tensor_tensor(qkT[0:ROT, sl], qkT[0:ROT, sl],
                                        tabc_sb[:, wsl], op=MUL)
                nc.vector.tensor_tensor(qkT[0:ROT, sl], qkT[0:ROT, sl],
                                        pr[0:ROT, :], op=ADD)

            # ---- QKV projection (single pass, all 4 heads) ----
            # rope matmuls are batched (not interleaved between full-K
            # chains) to minimize K-size transitions on the PE
            for n in range(NW):
                ht = ht0 if n == 0 else load_ht(n)
                for m in range(8):
                    pq = ps.tile([128, 512], F32, tag="v5", bufs=4,
                                 name=f"pq{m}_{n}")
                    for kc in range(KC):
                        nc.tensor.matmul(
                            pq[:],
                            wqk_sb[:, m * 2048 + kc * 128:m * 2048 + (kc + 1) * 128],
                            ht[:, kc * 512:(kc + 1) * 512],
                            start=(kc == 0), stop=(kc == KC - 1))
                    nc.vector.tensor_scalar_add(
                        qkT[:, m * S + n * 512:m * S + (n + 1) * 512],
                        pq[:], bqk_sb[:, m:m + 1])
                    if m > 0:
                        rope(m - 1, n)
                for t4 in range(4):
                    pv = ps.tile([128, 512], F32, tag="v5", bufs=4,
                                 name=f"pv{t4}_{n}")
                    for kc in range(KC):
                        nc.tensor.matmul(
                            pv[:],
                            ht[:, kc * 512 + t4 * 128:kc * 512 + (t4 + 1) * 128],
                            wv_sb[:, kc * 512:(kc + 1) * 512],
                            start=(kc == 0), stop=(kc == KC - 1))
                    tc4 = n * 4 + t4     # global 128-token chunk index
                    nc.vector.tensor_tensor(
                        vn.rearrange("p (h t) -> p h t", h=HPC)[
                            :, :, tc4 * 128:(tc4 + 1) * 128],
                        pv[:].rearrange("p (h t) -> p h t", h=HPC),
                        bv_sb.rearrange("p (h t) -> p h t", h=HPC),
                        op=ADD)
                    if t4 == 0:
                        rope(7, n)

            # ---- attention, software-pipelined TWO exps behind (the AV
            # matmuls of round k2 run while exp of k2+1 is in flight, so the
            # PE never waits on ACT latency) ----
            pending = []

            def consume(h, qs, k2, po, pm, e):
                for j in range(2):
                    kc = 2 * k2 + j
                    nc.tensor.matmul(po, vn[:, h * S + kc * 128:h * S + (kc + 1) * 128],
                                     e[:, j * 512:(j + 1) * 512],
                                     start=(kc == 0), stop=(kc == KC - 1))
                for j in range(2):
                    kc = 2 * k2 + j
                    nc.tensor.matmul(pm, ones_sb[:], e[:, j * 512:(j + 1) * 512],
                                     start=(kc == 0), stop=(kc == KC - 1))
                if k2 == KC // 2 - 1:
                    rc = epool.tile([128, 512], F32, tag="rc", bufs=2,
                                    name=f"rc{h}_{qs}")
                    nc.vector.reciprocal_approx_fast(rc[:], pm)
                    nc.vector.tensor_tensor(
                        oT[:, h * S + qs * 512:h * S + (qs + 1) * 512],
                        po, rc[:], op=MUL)

            def attention_head(h):
                qoff, koff = (2 * h) * S, (2 * h + 1) * S
                for qs in range(4):
                    po = ps.tile([128, 512], F32, tag="v5", bufs=4,
                                 name=f"po{h}_{qs}")
                    pm = ps.tile([128, 512], F32, tag="v5", bufs=4,
                                 name=f"pm{h}_{qs}")
                    for k2 in range(KC // 2):
                        pS = ps.tile([128, 1024], F32, tag="pS", bufs=2,
                                     name=f"pS{h}_{qs}_{k2}")
                        for j in range(2):
                            kc = 2 * k2 + j
                            nc.tensor.matmul(
                                pS[:, j * 512:(j + 1) * 512],
                                qkT[:, koff + kc * 128:koff + (kc + 1) * 128],
                                qkT[:, qoff + qs * 512:qoff + (qs + 1) * 512],
                                start=True, stop=True)
                        e = epool.tile([128, 1024], BF16, tag="e", bufs=4)
                        nc.scalar.activation(e[:], pS[:],
                                             mybir.ActivationFunctionType.Exp)
                        if len(pending) >= 2:
                            consume(*pending.pop(0))
                        pending.append((h, qs, k2, po, pm, e))
                if h == 0:
                    # dense weights: issued here so the transfer overlaps
                    # attention; needed only at the dense stage
                    nc.sync.dma_start(wd_sb[:], wd16)

            for h in range(HPC):
                attention_head(h)
            while pending:
                consume(*pending.pop(0))

            # ---- dense partial ----
            for tt in range(S // 128):
                for d2 in range(2):
                    bo = bopool.tile([128, 1024], F32, tag="bo",
                                     name=f"bo{tt}_{d2}")
                    for j in range(2):
                        dsp = 2 * d2 + j
                        pd = ps.tile([128, 512], F32, tag="v5", bufs=4,
                                     name=f"pd{tt}_{dsp}")
                        for hc in range(HPC):
                            nc.tensor.matmul(
                                pd[:],
                                oT[:, hc * S + tt * 128:hc * S + (tt + 1) * 128],
                                wd_sb[:, hc * D + dsp * 512:hc * D + (dsp + 1) * 512],
                                start=(hc == 0), stop=(hc == HPC - 1))
                        if j == 0:
                            nc.scalar.copy(bo[:, 0:512], pd[:])
                        else:
                            nc.vector.tensor_copy(bo[:, 512:1024], pd[:])
                    nc.sync.dma_start(
                        outp[tt * 128:(tt + 1) * 128,
                             d2 * 1024:(d2 + 1) * 1024],
                        bo[:])
    nc.compile()
    return nc


def _rope_tables(position_ids_b):
    pos = np.asarray(position_ids_b, dtype=np.float64)
    inv_freq = 1.0 / (BASE ** (np.arange(0, ROT, 2, dtype=np.float64) / ROT))
    freqs = np.outer(pos, inv_freq)                       # [S, 16]
    emb = np.concatenate([freqs, freqs], axis=-1)         # [S, 32]
    return (np.cos(emb).T.astype(NPBF).copy(),
            np.sin(emb).T.astype(NPBF).copy())


def kernel(hidden_states, position_ids, W_qkv, b_qkv, W_dense, b_dense):
    global _NC
    if _NC is None:
        _NC = _build()
    nc = _NC

    hidden_states = np.asarray(hidden_states, dtype=np.float32)
    W_qkv = np.asarray(W_qkv, dtype=np.float32)
    b_qkv = np.asarray(b_qkv, dtype=np.float32)
    W_dense = np.asarray(W_dense, dtype=np.float32)
    b_dense = np.asarray(b_dense, dtype=np.float32)

    rotm = np.zeros((128, ROT), np.float32)   # K zero-padded to 128
    half = ROT // 2
    for i in range(half):
        rotm[i + half, i] = -1.0
        rotm[i, i + half] = 1.0
    ones = np.ones((128, 128), np.float32)

    # hidden image: h_img[p, n*8192 + kc*512 + t] = hidden[b].T[kc*128+p, n*512+t]
    hTs = []
    for b in range(B):
        hT = hidden_states[b].T.reshape(KC, 128, NW, 512)
        hTs.append(np.ascontiguousarray(
            hT.transpose(1, 2, 0, 3).reshape(128, NW * KC * 512)).astype(NPBF))
    tabs_per_b = [_rope_tables(np.asarray(position_ids)[b]) for b in range(B)]

    in_maps = []
    for c in range(NCORES):
        b, g = divmod(c, CPB)
        # per-head column slices of W_qkv (NeoX fused layout: head-major,
        # [q(128) k(128) v(128)] per head)
        wqk = np.empty((D, 8 * 128), np.float32)
        wv = np.empty((D, HPC * 128), np.float32)
        bqk_host = np.empty((128, 8), np.float32)
        bv_host = np.empty((128, HPC * 128), np.float32)
        for hp in range(HPC):
            c0 = (g * HPC + hp) * 3 * HS
            wqk[:, (2 * hp) * 128:(2 * hp + 1) * 128] = \
                W_qkv[:, c0:c0 + HS] * SM_SCALE
            wqk[:, (2 * hp + 1) * 128:(2 * hp + 2) * 128] = \
                W_qkv[:, c0 + HS:c0 + 2 * HS]
            wv[:, hp * 128:(hp + 1) * 128] = W_qkv[:, c0 + 2 * HS:c0 + 3 * HS]
            bqk_host[:, 2 * hp] = b_qkv[c0:c0 + HS] * SM_SCALE
            bqk_host[:, 2 * hp + 1] = b_qkv[c0 + HS:c0 + 2 * HS]
            bv_host[:, hp * 128:(hp + 1) * 128] = \
                b_qkv[c0 + 2 * HS:c0 + 3 * HS][None, :]
        # SBUF-image layouts: [p, m*2048 + kc*128 + c], [p, kc*512 + c],
        # [p, hc*2048 + d]
        wqk_img = wqk.reshape(KC, 128, 8 * 128).transpose(1, 0, 2)  # p, kc, mc
        wqk_img = np.ascontiguousarray(
            wqk_img.reshape(128, KC, 8, 128).transpose(0, 2, 1, 3)
            .reshape(128, 8 * KC * 128))
        wv_img = np.ascontiguousarray(
            wv.reshape(KC, 128, HPC * 128).transpose(1, 0, 2)
            .reshape(128, KC * HPC * 128))
        wd_img = np.ascontiguousarray(
            W_dense[g * HPC * HS:(g + 1) * HPC * HS, :]
            .reshape(HPC, 128, D).transpose(1, 0, 2).reshape(128, HPC * D))
        cosT, sinT = tabs_per_b[b]
        in_maps.append({
            "h16d": hTs[b],
            "wqk16": wqk_img.astype(NPBF),
            "wv16": wv_img.astype(NPBF),
            "wd16": wd_img.astype(NPBF),
            "tabc16": cosT,
            "tabs16": sinT,
            "rotm16": rotm.astype(NPBF),
            "ones16": ones.astype(NPBF),
            "bqk": bqk_host,
            "bv": bv_host,
        })

    res = bass_utils.run_bass_kernel_spmd(
        nc, in_maps, core_ids=list(range(NCORES)), trace=TRACE)
    LAST_RESULT[0] = res

    out = np.empty((B, S, D), np.float32)
    for b in range(B):
        acc = np.zeros((S, D), np.float64)
        for g in range(CPB):
            acc += res.results[b * CPB + g]["outp"]
        out[b] = (acc + b_dense).astype(np.float32)
    return out


# revision 32
# speedup vs baseline: 1.2051x; 1.0173x over previous
"""Trainium2 Bass kernel for NeoX-style attention block (B=2, S=2048, D=2048,
H=16, HS=128, partial RoPE rot=32, no mask) sharded over 8 NeuronCores.

Sharding: core c handles batch b = c//4 and head group g = c%4 (4 heads).
Tensor-parallel over heads: W_qkv column-sliced, W_dense row-sliced; each core
produces a partial [S, D] output; host sums 4 partials per batch + bias.

All matmul operands are bf16 (f32 PSUM accumulation): identical PE rate to
f32r at N>=256 but halves DMA/SBUF, enables Fast Weight Load, and doubles DVE
throughput for 16-bit elementwise ops.  Single QKV pass (weights resident).

Per-core dataflow:
  q,k produced transposed (qkT[hs, tok]) via lhsT = W chunks, rhs = hT chunks;
  V produced NATURAL ([tok, hs]) via the swapped matmul lhsT = hT chunk,
  rhs = Wv (kills the per-head PE transposes of V).  Partial RoPE (first 32
  rows of each q/k chunk) via a rotate-half matmul + 3 DVE ops per window.
  Scores^T chunks S^T[k, q] = K^T_chunk.T @ Q^T (512-wide), exp on ACT
  (bf16 out), AV accum O^T = V_chunk.T @ E plus a ones-matmul accumulating
  softmax denominators; normalization = reciprocal_approx_fast + multiply.
  Dense partial uses lhsT = O^T chunks, rhs = W_dense row-slice.
"""
import sys

sys.path.insert(0, "/opt/trn_rl_repo")

import numpy as np
import ml_dtypes
from contextlib import ExitStack

import concourse.bass as bass  # noqa: F401  (registers engine types)
import concourse.tile as tile
from concourse import bacc, mybir
from concourse import bass_utils

F32 = mybir.dt.float32
BF16 = mybir.dt.bfloat16
NPBF = ml_dtypes.bfloat16
MUL = mybir.AluOpType.mult
ADD = mybir.AluOpType.add

B, S, D = 2, 2048, 2048
H, HS, ROT = 16, 128, 32
BASE = 10000.0
SM_SCALE = 1.0 / float(np.sqrt(HS))

HPC = 4            # heads per core
CPB = 4            # cores per batch
NCORES = 8
KC = D // 128      # 16 contraction chunks
NW = 4             # token windows of 512

_NC = None
TRACE = False
LAST_RESULT = [None]


def _build():
    nc = bacc.Bacc("TRN2", target_bir_lowering=False, debug=False)
    # all big inputs are host-prepared in the exact SBUF image layout so each
    # DMA is contiguous per partition (128 fat descriptors, not 2048 thin)
    h16d = nc.dram_tensor("h16d", [128, NW * KC * 512], BF16,
                          kind="ExternalInput").ap()
    wqk16 = nc.dram_tensor("wqk16", [128, 8 * KC * 128], BF16,
                           kind="ExternalInput").ap()
    wv16 = nc.dram_tensor("wv16", [128, KC * 512], BF16,
                          kind="ExternalInput").ap()
    wd16 = nc.dram_tensor("wd16", [128, HPC * D], BF16,
                          kind="ExternalInput").ap()
    tabc16 = nc.dram_tensor("tabc16", [ROT, S], BF16, kind="ExternalInput").ap()
    tabs16 = nc.dram_tensor("tabs16", [ROT, S], BF16, kind="ExternalInput").ap()
    rotm16 = nc.dram_tensor("rotm16", [128, 128], BF16, kind="ExternalInput").ap()
    ones16 = nc.dram_tensor("ones16", [128, 128], BF16, kind="ExternalInput").ap()
    bqk = nc.dram_tensor("bqk", [128, 8], F32, kind="ExternalInput").ap()
    bv = nc.dram_tensor("bv", [128, HPC * 128], F32, kind="ExternalInput").ap()
    outp = nc.dram_tensor("outp", [S, D], F32, kind="ExternalOutput").ap()

    with tile.TileContext(nc) as tc:
        with ExitStack() as ctx:
            glob = ctx.enter_context(tc.tile_pool(name="glob", bufs=1))
            hpool = ctx.enter_context(tc.tile_pool(name="hpool", bufs=2))
            epool = ctx.enter_context(tc.tile_pool(name="epool", bufs=3))
            bopool = ctx.enter_context(tc.tile_pool(name="bopool", bufs=4))
            ps = ctx.enter_context(tc.tile_pool(name="ps", bufs=1, space="PSUM"))

            # ---- resident weights / tables ----
            # DMA issue order is critical for startup latency: first QKV
            # weight chunk + first token window lead; dense weights trail.
            wqk_sb = glob.tile([128, 8 * KC * 128], BF16, tag="wqk")

            def load_wqk(m):
                nc.sync.dma_start(
                    wqk_sb[:, m * 2048:(m + 1) * 2048],
                    wqk16[:, m * 2048:(m + 1) * 2048])

            load_wqk(0)

            def load_ht(n, engines=None):
                ht = hpool.tile([128, KC * 512], BF16, tag="ht",
                                name=f"ht{n}")
                engines = engines or [nc.sync]
                w = KC * 512 // len(engines)
                for k4, eng in enumerate(engines):
                    eng.dma_start(
                        ht[:, k4 * w:(k4 + 1) * w],
                        h16d[:, n * KC * 512 + k4 * w:
                             n * KC * 512 + (k4 + 1) * w])
                return ht

            # first window: issue from otherwise-idle engine queues in
            # parallel with wqk on sync, to cut the serialized-issue head
            ht0 = load_ht(0, engines=[nc.scalar] * 4)

            # PE warmup: dummy matmuls on a never-written SBUF tile overlap
            # the initial DMA head and bring the PE clock to full speed
            # before the first real matmul (results are never read)
            junk = glob.tile([128, 512], BF16, tag="junk")
            nc.vector.memset(junk[:], 0.0)
            warm = ps.tile([128, 512], F32, tag="v5", bufs=4, name="warm")
            for _ in range(24):
                nc.tensor.matmul(warm[:], junk[:, 0:128], junk[:],
                                 start=True, stop=True)
            load_wqk(1)
            tabc_sb = glob.tile([ROT, S], BF16, tag="tabc")
            nc.sync.dma_start(tabc_sb[:], tabc16)
            tabs_sb = glob.tile([ROT, S], BF16, tag="tabs")
            nc.sync.dma_start(tabs_sb[:], tabs16)
            # rotate-half matrix zero-padded to K=M=128 so the rope matmul
            # keeps full array dims (thin-K/thin-M matmuls trigger HAM clock
            # oscillation that slows the surrounding matmuls)
            rot_sb = glob.tile([128, 128], BF16, tag="rotm")
            nc.sync.dma_start(rot_sb[:], rotm16)
            bqk_sb = glob.tile([128, 8], F32, tag="bqk")
            nc.sync.dma_start(bqk_sb[:], bqk)
            for m in range(2, 8):
                load_wqk(m)
            wv_sb = glob.tile([128, KC * 512], BF16, tag="wv")
            nc.sync.dma_start(wv_sb[:], wv16)
            bv_sb = glob.tile([128, HPC * 128], F32, tag="bv")
            nc.sync.dma_start(bv_sb[:], bv)
            ones_sb = glob.tile([128, 128], BF16, tag="ones")
            nc.sync.dma_start(ones_sb[:], ones16)

            # persistent activations
            qkT = glob.tile([128, 8 * S], BF16, tag="qkT")   # [hs, m*S + tok]
            vn = glob.tile([128, HPC * S], BF16, tag="vn")   # [ktok, h*S+kc*128+hs]
            oT = glob.tile([128, HPC * S], BF16, tag="oT")   # [hs, h*S + tok]
            wd_sb = glob.tile([128, HPC * D], BF16, tag="wd")

            def rope(m, n):
                # rotate first ROT dims of q/k chunk m for token window n.
                # Issued one m-chunk late so the rot matmul's dependency (the
                # DVE eviction of chunk m) is already done when PE reaches it.
                sl = slice(m * S + n * 512, m * S + (n + 1) * 512)
                wsl = slice(n * 512, (n + 1) * 512)
                pr = ps.tile([128, 512], F32, tag="v5", bufs=4, name=f"pr{m}_{n}")
                nc.tensor.matmul(pr[:], rot_sb[:, :],
                                 qkT[:, sl], start=True, stop=True)
                nc.vector.tensor_tensor(pr[0:ROT, :], pr[0:ROT, :],
                                        tabs_sb[:, wsl], op=MUL)
                nc.vector.tensor_tensor(qkT[0:ROT, sl], qkT[0:ROT, sl],
                                        tabc_sb[:, wsl], op=MUL)
                nc.vector.tensor_tensor(qkT[0:ROT, sl], qkT[0:ROT, sl],
                                        pr[0:ROT, :], op=ADD)

            # ---- QKV projection (single pass, all 4 heads) ----
            # rope matmuls are batched (not interleaved between full-K
            # chains) to minimize K-size transitions on the PE
            for n in range(NW):
                ht = ht0 if n == 0 else load_ht(n)
                for m in range(8):
                    pq = ps.tile([128, 512], F32, tag="v5", bufs=4,
                                 name=f"pq{m}_{n}")
                    for kc in range(KC):
                        nc.tensor.matmul(
                            pq[:],
                            wqk_sb[:, m * 2048 + kc * 128:m * 2048 + (kc + 1) * 128],
                            ht[:, kc * 512:(kc + 1) * 512],
                            start=(kc == 0), stop=(kc == KC - 1))
                    nc.vector.tensor_scalar_add(
                        qkT[:, m * S + n * 512:m * S + (n + 1) * 512],
                        pq[:], bqk_sb[:, m:m + 1])
                    if m > 0:
                        rope(m - 1, n)
                for t4 in range(4):
                    pv = ps.tile([128, 512], F32, tag="v5", bufs=4,
                                 name=f"pv{t4}_{n}")
                    for kc in range(KC):
                        nc.tensor.matmul(
                            pv[:],
                            ht[:, kc * 512 + t4 * 128:kc * 512 + (t4 + 1) * 128],
                            wv_sb[:, kc * 512:(kc + 1) * 512],
                            start=(kc == 0), stop=(kc == KC - 1))
                    tc4 = n * 4 + t4     # global 128-token chunk index
                    nc.vector.tensor_tensor(
                        vn.rearrange("p (h t) -> p h t", h=HPC)[
                            :, :, tc4 * 128:(tc4 + 1) * 128],
                        pv[:].rearrange("p (h t) -> p h t", h=HPC),
                        bv_sb.rearrange("p (h t) -> p h t", h=HPC),
                        op=ADD)
                    if t4 == 0:
                        rope(7, n)

            # ---- attention, software-pipelined TWO exps behind (the AV
            # matmuls of round k2 run while exp of k2+1 is in flight, so the
            # PE never waits on ACT latency) ----
            pending = []

            def consume(h, qs, k2, po, pm, e):
                for j in range(2):
                    kc = 2 * k2 + j
                    nc.tensor.matmul(po, vn[:, h * S + kc * 128:h * S + (kc + 1) * 128],
                                     e[:, j * 512:(j + 1) * 512],
                                     start=(kc == 0), stop=(kc == KC - 1))
                for j in range(2):
                    kc = 2 * k2 + j
                    nc.tensor.matmul(pm, ones_sb[:], e[:, j * 512:(j + 1) * 512],
                                     start=(kc == 0), stop=(kc == KC - 1))
                if k2 == KC // 2 - 1:
                    rc = epool.tile([128, 512], F32, tag="rc", bufs=2,
                                    name=f"rc{h}_{qs}")
                    nc.vector.reciprocal_approx_fast(rc[:], pm)
                    nc.vector.tensor_tensor(
                        oT[:, h * S + qs * 512:h * S + (qs + 1) * 512],
                        po, rc[:], op=MUL)

            def attention_head(h):
                qoff, koff = (2 * h) * S, (2 * h + 1) * S
                for qs in range(4):
                    po = ps.tile([128, 512], F32, tag="v5", bufs=4,
                                 name=f"po{h}_{qs}")
                    pm = ps.tile([128, 512], F32, tag="v5", bufs=4,
                                 name=f"pm{h}_{qs}")
                    for k2 in range(KC // 2):
                        pS = ps.tile([128, 1024], F32, tag="pS", bufs=2,
                                     name=f"pS{h}_{qs}_{k2}")
                        for j in range(2):
                            kc = 2 * k2 + j
                            nc.tensor.matmul(
                                pS[:, j * 512:(j + 1) * 512],
                                qkT[:, koff + kc * 128:koff + (kc + 1) * 128],
                                qkT[:, qoff + qs * 512:qoff + (qs + 1) * 512],
                                start=True, stop=True)
                        e = epool.tile([128, 1024], BF16, tag="e", bufs=4)
                        nc.scalar.activation(e[:], pS[:],
                                             mybir.ActivationFunctionType.Exp)
                        if len(pending) >= 2:
                            consume(*pending.pop(0))
                        pending.append((h, qs, k2, po, pm, e))
                if h == 0:
                    # dense weights: issued here so the transfer overlaps
                    # attention; needed only at the dense stage
                    nc.sync.dma_start(wd_sb[:], wd16)

            for h in range(HPC):
                attention_head(h)
            while pending:
                consume(*pending.pop(0))

            # ---- dense partial ----
            for tt in range(S // 128):
                for d2 in range(2):
                    bo = bopool.tile([128, 1024], F32, tag="bo",
                                     name=f"bo{tt}_{d2}")
                    for j in range(2):
                        dsp = 2 * d2 + j
                        pd = ps.tile([128, 512], F32, tag="v5", bufs=4,
                                     name=f"pd{tt}_{dsp}")
                        for hc in range(HPC):
                            nc.tensor.matmul(
                                pd[:],
                                oT[:, hc * S + tt * 128:hc * S + (tt + 1) * 128],
                                wd_sb[:, hc * D + dsp * 512:hc * D + (dsp + 1) * 512],
                                start=(hc == 0), stop=(hc == HPC - 1))
                        if j == 0:
                            nc.scalar.copy(bo[:, 0:512], pd[:])
                        else:
                            nc.vector.tensor_copy(bo[:, 512:1024], pd[:])
                    nc.sync.dma_start(
                        outp[tt * 128:(tt + 1) * 128,
                             d2 * 1024:(d2 + 1) * 1024],
                        bo[:])
    nc.compile()
    return nc


def _rope_tables(position_ids_b):
    pos = np.asarray(position_ids_b, dtype=np.float64)
    inv_freq = 1.0 / (BASE ** (np.arange(0, ROT, 2, dtype=np.float64) / ROT))
    freqs = np.outer(pos, inv_freq)                       # [S, 16]
    emb = np.concatenate([freqs, freqs], axis=-1)         # [S, 32]
    return (np.cos(emb).T.astype(NPBF).copy(),
            np.sin(emb).T.astype(NPBF).copy())


def kernel(hidden_states, position_ids, W_qkv, b_qkv, W_dense, b_dense):
    global _NC
    if _NC is None:
        _NC = _build()
    nc = _NC

    hidden_states = np.asarray(hidden_states, dtype=np.float32)
    W_qkv = np.asarray(W_qkv, dtype=np.float32)
    b_qkv = np.asarray(b_qkv, dtype=np.float32)
    W_dense = np.asarray(W_dense, dtype=np.float32)
    b_dense = np.asarray(b_dense, dtype=np.float32)

    rotm = np.zeros((128, 128), np.float32)   # K and M zero-padded to 128
    half = ROT // 2
    for i in range(half):
        rotm[i + half, i] = -1.0
        rotm[i, i + half] = 1.0
    ones = np.ones((128, 128), np.float32)

    # hidden image: h_img[p, n*8192 + kc*512 + t] = hidden[b].T[kc*128+p, n*512+t]
    hTs = []
    for b in range(B):
        hT = hidden_states[b].T.reshape(KC, 128, NW, 512)
        hTs.append(np.ascontiguousarray(
            hT.transpose(1, 2, 0, 3).reshape(128, NW * KC * 512)).astype(NPBF))
    tabs_per_b = [_rope_tables(np.asarray(position_ids)[b]) for b in range(B)]

    in_maps = []
    for c in range(NCORES):
        b, g = divmod(c, CPB)
        # per-head column slices of W_qkv (NeoX fused layout: head-major,
        # [q(128) k(128) v(128)] per head)
        wqk = np.empty((D, 8 * 128), np.float32)
        wv = np.empty((D, HPC * 128), np.float32)
        bqk_host = np.empty((128, 8), np.float32)
        bv_host = np.empty((128, HPC * 128), np.float32)
        for hp in range(HPC):
            c0 = (g * HPC + hp) * 3 * HS
            wqk[:, (2 * hp) * 128:(2 * hp + 1) * 128] = \
                W_qkv[:, c0:c0 + HS] * SM_SCALE
            wqk[:, (2 * hp + 1) * 128:(2 * hp + 2) * 128] = \
                W_qkv[:, c0 + HS:c0 + 2 * HS]
            wv[:, hp * 128:(hp + 1) * 128] = W_qkv[:, c0 + 2 * HS:c0 + 3 * HS]
            bqk_host[:, 2 * hp] = b_qkv[c0:c0 + HS] * SM_SCALE
            bqk_host[:, 2 * hp + 1] = b_qkv[c0 + HS:c0 + 2 * HS]
            bv_host[:, hp * 128:(hp + 1) * 128] = \
                b_qkv[c0 + 2 * HS:c0 + 3 * HS][None, :]
        # SBUF-image layouts: [p, m*2048 + kc*128 + c], [p, kc*512 + c],
        # [p, hc*2048 + d]
        wqk_img = wqk.reshape(KC, 128, 8 * 128).transpose(1, 0, 2)  # p, kc, mc
        wqk_img = np.ascontiguousarray(
            wqk_img.reshape(128, KC, 8, 128).transpose(0, 2, 1, 3)
            .reshape(128, 8 * KC * 128))
        wv_img = np.ascontiguousarray(
            wv.reshape(KC, 128, HPC * 128).transpose(1, 0, 2)
            .reshape(128, KC * HPC * 128))
        wd_img = np.ascontiguousarray(
            W_dense[g * HPC * HS:(g + 1) * HPC * HS, :]
            .reshape(HPC, 128, D).transpose(1, 0, 2).reshape(128, HPC * D))
        cosT, sinT = tabs_per_b[b]
        in_maps.append({
            "h16d": hTs[b],
            "wqk16": wqk_img.astype(NPBF),
            "wv16": wv_img.astype(NPBF),
            "wd16": wd_img.astype(NPBF),
            "tabc16": cosT,
            "tabs16": sinT,
            "rotm16": rotm.astype(NPBF),
            "ones16": ones.astype(NPBF),
            "bqk": bqk_host,
            "bv": bv_host,
        })

    res = bass_utils.run_bass_kernel_spmd(
        nc, in_maps, core_ids=list(range(NCORES)), trace=TRACE)
    LAST_RESULT[0] = res

    out = np.empty((B, S, D), np.float32)
    for b in range(B):
        acc = np.zeros((S, D), np.float64)
        for g in range(CPB):
            acc += res.results[b * CPB + g]["outp"]
        out[b] = (acc + b_dense).astype(np.float32)
    return out
